# revision 12
# baseline (speedup 1.0000x reference)
"""AdaptedAttention (llama + adaption-prompt) on 8 TRN2 NeuronCores.

Sharding: tensor-parallel over heads (2 heads/core), zero device collectives.
Per core (everything on-chip fp16; PSUM accumulation fp32):
  - qT/kT/V projections for its 2 heads from fp16, pre-transposed X
    (all matmuls contract over d, so X lives on-chip as X.T [d part, s free]).
    The q/k projection t-loop is OUTERMOST (4 open PSUM accumulation groups:
    q+k x 2 heads) so chunk-0's PE consumption order matches the startup DMA
    arrival order t-slice by t-slice -- PE starts ~1 us after launch and
    streams at the DMA rate instead of stalling on late weight slices.
  - RoPE in the [hd, s] layout with HOST-precomputed fp16 cos/sin tables
    (sin pre-signed AND half-swapped so each DVE TensorTensor's two SBUF
    inputs share a base partition -- a HW requirement)
  - attention as S^T = K @ Q^T tiles ([k part, q free]) so softmax needs no
    transposes. Score k-tiles are computed in PAIRS into one [128, 2, QC]
    fp32 PSUM tile (2 banks) so ONE ACT exp covers 1024 columns -- the ACT
    per-op init (~185 ns) amortizes over 2 tiles, which matters because the
    k-loop is ACT-throughput-bound. Causal handling: skip k-tiles above the
    diagonal, col-restricted ctx/acc reads on diagonal tiles (the below-diag
    exp garbage is never read), and one [128,128] triangular fp16 mask for
    the 128-wide mixed strip of each diagonal tile. Row sums: est tiles
    accumulate elementwise on the DVE (fp16 2x), then ONE ones-matmul
    partition-reduce per head into a shared [97, QC] sums bank (rows 0/32/
    64/96: main/adapter x head -- matmul outputs need 32-aligned bases).
  - softmax denominators are read back (DVE reciprocals) IMMEDIATELY after
    each head's k-loop: the sums bank shares a pool with out-proj PSUM
    tiles, and freeing it early unblocks the next chunk's allocations.
  - adapter path (L=10) folded into the main ctx PSUM accumulation:
    ct = (ctx + actx_g * sum/asum) / sum, gate pre-folded into the
    host-scaled adapter-V copy (aptv). 1/asum stays fp32 (asum ~ 6e5 makes
    it fp16-subnormal). The [1,QC] -> [L,QC] and [1,QC] -> [128,QC]
    broadcasts run on the otherwise-idle GPSIMD engine (partition_broadcast)
    instead of burning PE matmuls + ACT copies.
  - output projection uses only the core's OWN 2-head ct against its
    256-column slice of Wo, producing fp16 partials [d, s]; the cross-core
    sum happens on the host as the unshard step. out_proj(qc) is NOT
    emitted right after chunk qc: it is deferred and interleaved, one
    dt-tile at a time, into chunk qc+1's k-loop emission, so the PE has
    filler work during the ACT-bound stretches of the score loop (the PE
    engine queue is FIFO -- anything emitted later cannot jump ahead).
    In the LAST chunk, head 0's combine chain is likewise deferred into
    head 1's k-loop (hiding its DVE/Pool latency under PE score work), and
    the final out-proj ping-pongs over FOUR PSUM slots (2 pools) with a
    2-ahead first-head window so the tail runs at PE rate, not at
    PSUM-evacuation-copy latency.
PSUM banks (8): big x2 (score-pairs / merged q+k proj / v proj / adapter
projections / tail out-proj; [128,2,QC] fp32 slots = 2 banks each), ctx x2,
accps x2 (sums / adapter scores / out-proj accumulators).
Host side: weight slicing/transposes/casts, RoPE tables from position_ids,
partial-sum + transpose.
"""

import math
import numpy as np

import concourse.bass as bass
import concourse.bacc as bacc
import concourse.mybir as mybir
import concourse.tile as tile
from concourse.bass_utils import run_bass_kernel_spmd

F16 = mybir.dt.float16
BF16 = mybir.dt.bfloat16
F32 = mybir.dt.float32
NP_F16 = mybir.dt.np(F16)
NP_BF16 = mybir.dt.np(BF16)


class Cfg:
    def __init__(self, s=2048, d=2048, L=10, n_cores=8, n_heads=16, rope_base=10000.0):
        self.s, self.d, self.L = s, d, L
        self.n_cores = n_cores
        self.n_heads = n_heads
        self.rope_base = rope_base
        self.hd = 128                      # head dim (fixed)
        self.hpc = n_heads // n_cores      # heads per core
        self.dh = self.hpc * self.hd       # local head-dim cols per core
        self.nd = d // 128                 # contraction chunks
        self.QC = 512                      # q-chunk width
        self.ns = s // self.QC             # q-chunks
        self.nst = s // 128                # s tiles (k tiles)
        self.kpq = self.QC // 128          # k-tiles straddling one q-chunk diag
        assert self.hpc * n_cores == n_heads and d % 128 == 0 and s % self.QC == 0
        assert self.kpq % 2 == 0


def build(cfg: Cfg, nrep: int = 1):
    """Build the per-core SPMD graph. Returns compiled nc.
    nrep>1 repeats the whole pipeline (for marginal-time HW measurement)."""
    c = cfg
    nc = bacc.Bacc(None, target_bir_lowering=False, num_devices=c.n_cores)

    # ---------------- external I/O (per-core shards) ----------------
    xt_d = nc.dram_tensor("xt", [c.d, c.s], F16, kind="ExternalInput")
    wqk_d = nc.dram_tensor("wqk", [c.d, 2, c.dh], F16, kind="ExternalInput")
    wvt_d = nc.dram_tensor("wvt", [c.d, c.dh], F16, kind="ExternalInput")
    wot_d = nc.dram_tensor("wot", [c.dh, c.d], F16, kind="ExternalInput")
    apt_d = nc.dram_tensor("apt", [c.d, c.L], F16, kind="ExternalInput")
    aptv_d = nc.dram_tensor("aptv", [c.d, c.L], F16, kind="ExternalInput")
    cos_d = nc.dram_tensor("cost", [128, c.s], F16, kind="ExternalInput")
    sin_d = nc.dram_tensor("sint", [128, c.s], F16, kind="ExternalInput")
    out_d = nc.dram_tensor("out", [c.d, c.s], F16, kind="ExternalOutput")

    # single [128,128] lower-triangular mask: tri[k, q] = 1 if k <= q.
    # Diagonal k-tile j of a chunk only mixes masked/unmasked inside a
    # 128-wide column strip; columns below it are handled by col-restricted
    # reads, columns above are fully unmasked.
    kk = np.arange(128)[:, None]
    qq = np.arange(128)[None, :]
    tri_np = (kk <= qq).astype(NP_F16)  # [128, 128]
    tri_d = nc.inline_tensor(tri_np, name="tri")

    scale_s = 1.0 / math.sqrt(c.hd)        # main attention scale
    scale_a = 1.0 / math.sqrt(c.n_heads)   # adapter scale (faithful to ref)

    EXP = mybir.ActivationFunctionType.Exp
    ADD = mybir.AluOpType.add
    MUL = mybir.AluOpType.mult

    with tile.TileContext(nc) as tc:
        with (
            tc.tile_pool(name="bigsb", bufs=1) as bigp,
            tc.tile_pool(name="persist", bufs=1) as pp,
            tc.tile_pool(name="work", bufs=3) as wp,
            tc.tile_pool(name="psum", bufs=1, space="PSUM") as psp,
        ):
            if nrep == 0:
                # timing baseline: touch every input (the terminal only ships
                # buffers the NEFF references) but do ~zero device work
                z = pp.tile([1, 128], F16, tag="z")
                for i, t in enumerate((xt_d, wqk_d, wvt_d, wot_d,
                                       apt_d, aptv_d, cos_d, sin_d)):
                    nc.sync.dma_start(z[0:1, 8 * i:8 * i + 8], t[0:1, 0:8])
                zo = pp.tile([1, 64], F16, tag="zo")
                nc.gpsimd.memset(zo[:], 0.0)
                nc.sync.dma_start(out_d[0:1, 0:64], zo[:])
            for _rep in range(nrep):
                # ---------- loads (q/k weights + chunk0, t-sliced, first) ----
                # q and k weights interleaved per t-slice so one DMA
                # stream delivers them in exactly the consumption order
                wqk = pp.tile([128, c.nd, 2, c.dh], F16, tag="wqk")
                wqk_r = wqk_d.rearrange("(t p) two m -> p t two m", p=128)
                xt = bigp.tile([128, c.nd, c.s], F16, tag="big")
                xt_r = xt_d.rearrange("(t p) s -> p t s", p=128)
                # t-sliced startup loads, arrival order == the t-interleaved
                # consumption order of proj_chunk(0): wq/wk ride the SP HWDGE
                # ring while chunk-0 x slices ride the otherwise-idle ACT ring
                bounds = [b for b in (0, 1, 3, 6, 10, 13, c.nd) if b <= c.nd]
                bounds = sorted(set(bounds + [c.nd]))
                cos_t = pp.tile([128, c.s], F16, tag="cos")
                sin_t = pp.tile([128, c.s], F16, tag="sin")
                for i in range(len(bounds) - 1):
                    ts = slice(bounds[i], bounds[i + 1])
                    nc.sync.dma_start(wqk[:, ts, :, :], wqk_r[:, ts, :, :])
                    nc.scalar.dma_start(xt[:, ts, 0:c.QC], xt_r[:, ts, 0:c.QC])
                nc.sync.dma_start(cos_t[:, 0:c.QC], cos_d[:, 0:c.QC])
                nc.sync.dma_start(sin_t[:, 0:c.QC], sin_d[:, 0:c.QC])
                # dummy exp: pulls the ACT function-table load into the
                # startup DMA window instead of stalling the first real copy
                warm = wp.tile([1, 2], F32, tag="warm", bufs=1)
                nc.vector.memset(warm[:], 0.0)
                nc.scalar.activation(warm[:], warm[:], EXP, scale=1.0)
                tri = pp.tile([128, 128], F16, tag="tri")
                nc.sync.dma_start(tri[:], tri_d[:])
                # adapter inputs are tiny (80 KB) but feed adapter_kv early in
                # the PE queue -- they must not trail the 12 MB of bulk loads
                apt = pp.tile([128, c.nd, c.L], F16, tag="apt")
                nc.sync.dma_start(apt[:], apt_d.rearrange("(t p) m -> p t m", p=128))
                aptv = pp.tile([128, c.nd, c.L], F16, tag="aptv")
                nc.sync.dma_start(aptv[:], aptv_d.rearrange("(t p) m -> p t m", p=128))
                wvt = pp.tile([128, c.nd, c.dh], F16, tag="wvt")
                nc.sync.dma_start(wvt[:], wvt_d.rearrange("(t p) m -> p t m", p=128))
                if c.s > c.QC:
                    sl = slice(c.QC, c.s)
                    nc.sync.dma_start(cos_t[:, sl], cos_d[:, sl])
                    nc.sync.dma_start(sin_t[:, sl], sin_d[:, sl])
                for qc in range(1, c.ns):
                    sl = slice(qc * c.QC, (qc + 1) * c.QC)
                    nc.sync.dma_start(xt[:, :, sl], xt_r[:, :, sl])
                wot = pp.tile([128, c.hpc, c.d], F16, tag="wot")
                nc.sync.dma_start(wot[:], wot_d.rearrange("(t p) m -> p t m", p=128))
                # all-ones: column [:, 0:1] is the row-sum lhsT
                ones_t = pp.tile([128, 128], F16, tag="ones_t")
                nc.gpsimd.memset(ones_t[:], 1.0)

                # ---------- persistent intermediates ----------
                qrot = [pp.tile([128, c.s], F16, tag=f"qrot{h}", name=f"qrot{h}")
                        for h in range(c.hpc)]
                krot = [pp.tile([128, c.s], F16, tag=f"krot{h}", name=f"krot{h}")
                        for h in range(c.hpc)]
                v_sb = pp.tile([128, c.nst, c.dh], F16, tag="v")
                akt = pp.tile([128, c.hpc, c.L], F16, tag="akt")
                av_sb = pp.tile([c.L, c.dh], F16, tag="av")

                def rope_copy(src_ps, on_act):
                    # PSUM -> SBUF evacuation, FIRST so the projection bank
                    # frees after one short copy instead of a full TT chain;
                    # half the copies ride the otherwise-idle ACT engine
                    qf = wp.tile([128, c.QC], F16, tag="qf", bufs=4)
                    if on_act:
                        nc.scalar.copy(qf[:], src_ps[:])
                    else:
                        nc.vector.tensor_copy(qf[:], src_ps[:])
                    return qf

                def rope_tt(dst, qf, sl):
                    # dst[0:64]   = src[0:64]*cos[0:64] - src[64:]*sin[0:64]
                    # dst[64:128] = src[64:]*cos[64:]   + src[0:64]*sin[64:]
                    # sin_t is pre-signed on host: rows 0:64 hold -sin, and
                    # halves are swapped so each TT's two SBUF inputs share a
                    # base partition (HW requirement)
                    t2 = wp.tile([128, c.QC], F16, tag="tmp", bufs=6)
                    nc.vector.tensor_tensor(t2[0:64], qf[64:128],
                                            sin_t[64:128, sl], MUL)
                    nc.vector.tensor_tensor(t2[64:128], qf[0:64],
                                            sin_t[0:64, sl], MUL)
                    t1 = wp.tile([128, c.QC], F16, tag="tmp", bufs=6)
                    nc.vector.tensor_tensor(t1[:], qf[:], cos_t[:, sl], MUL)
                    nc.vector.tensor_tensor(dst, t1[:], t2[:], ADD)

                def proj_chunk(qc):
                    sl = slice(qc * c.QC, (qc + 1) * c.QC)
                    if qc == 0:
                        # t OUTERMOST (4 open accumulation groups: q+k x 2
                        # heads) so chunk-0 consumption matches the startup
                        # DMA arrival order t-slice by t-slice
                        qk_ps = [psp.tile([128, 2, c.QC], F32, tag="big",
                                          bufs=2, name=f"qk{qc}_{h}")
                                 for h in range(c.hpc)]
                        for t in range(c.nd):
                            for h in range(c.hpc):
                                hsl = slice(h * 128, (h + 1) * 128)
                                nc.tensor.matmul(qk_ps[h][:, 0, :],
                                                 wqk[:, t, 0, hsl], xt[:, t, sl],
                                                 start=(t == 0),
                                                 stop=(t == c.nd - 1))
                                nc.tensor.matmul(qk_ps[h][:, 1, :],
                                                 wqk[:, t, 1, hsl], xt[:, t, sl],
                                                 start=(t == 0),
                                                 stop=(t == c.nd - 1))
                        qfs = []
                        for h in range(c.hpc):
                            qfs.append(rope_copy(qk_ps[h][:, 0, :], False))
                            qfs.append(rope_copy(qk_ps[h][:, 1, :], True))
                        for h in range(c.hpc):
                            rope_tt(qrot[h][:, sl], qfs[2 * h], sl)
                            rope_tt(krot[h][:, sl], qfs[2 * h + 1], sl)
                    else:
                        for h in range(c.hpc):
                            hsl = slice(h * 128, (h + 1) * 128)
                            qk_ps = psp.tile([128, 2, c.QC], F32, tag="big",
                                             bufs=2, name=f"qk{qc}_{h}")
                            for t in range(c.nd):
                                nc.tensor.matmul(qk_ps[:, 0, :], wqk[:, t, 0, hsl],
                                                 xt[:, t, sl], start=(t == 0),
                                                 stop=(t == c.nd - 1))
                            for t in range(c.nd):
                                nc.tensor.matmul(qk_ps[:, 1, :], wqk[:, t, 1, hsl],
                                                 xt[:, t, sl], start=(t == 0),
                                                 stop=(t == c.nd - 1))
                            qf_q = rope_copy(qk_ps[:, 0, :], False)
                            qf_k = rope_copy(qk_ps[:, 1, :], True)
                            rope_tt(qrot[h][:, sl], qf_q, sl)
                            rope_tt(krot[h][:, sl], qf_k, sl)
                    for st in range(c.kpq):
                        gst = qc * c.kpq + st
                        ssl = slice(gst * 128, (gst + 1) * 128)
                        v_ps = psp.tile([128, c.dh], F32, tag="big", bufs=2,
                                        name=f"v{qc}_{st}")
                        for t in range(c.nd):
                            nc.tensor.matmul(v_ps[:], xt[:, t, ssl], wvt[:, t, :],
                                             start=(t == 0), stop=(t == c.nd - 1))
                        nc.scalar.copy(v_sb[:, gst, :], v_ps[:])

                def adapter_kv():
                    for h in range(c.hpc):
                        hsl = slice(h * 128, (h + 1) * 128)
                        a_ps = psp.tile([128, c.L], F32, tag="big", bufs=2)
                        for t in range(c.nd):
                            nc.tensor.matmul(a_ps[:], wqk[:, t, 1, hsl], apt[:, t, :],
                                             start=(t == 0), stop=(t == c.nd - 1))
                        nc.scalar.copy(akt[:, h, :], a_ps[:])
                    av_ps = psp.tile([c.L, c.dh], F32, tag="big", bufs=2)
                    for t in range(c.nd):
                        nc.tensor.matmul(av_ps[:], aptv[:, t, :], wvt[:, t, :],
                                         start=(t == 0), stop=(t == c.nd - 1))
                    nc.scalar.copy(av_sb[:], av_ps[:])

                def scores_head(qc, h, state, filler, sums_box):
                    sl = slice(qc * c.QC, (qc + 1) * c.QC)
                    nkt = qc * c.kpq + c.kpq  # causal: k-tiles 0..nkt-1
                    npair = nkt // 2
                    hsl = slice(h * 128, (h + 1) * 128)
                    ctx_ps = psp.tile([128, c.QC], F32, tag="ctx", bufs=2,
                                      name=f"ctx{qc}_{h}")

                    def pair_mm(m):
                        # two k-tiles of S^T into one 2-bank fp32 tile so
                        # the exp below covers 1024 columns in one op; on
                        # diagonal tiles only the live columns are computed
                        ps = psp.tile([128, 2, c.QC], F32, tag="big", bufs=2,
                                      name=f"st{qc}_{h}_{m}")
                        for i in (0, 1):
                            kt = 2 * m + i
                            j = kt - qc * c.kpq
                            lo = 128 * j if j > 0 else 0
                            ksl = slice(kt * 128, (kt + 1) * 128)
                            nc.tensor.matmul(
                                ps[:, i, lo:], krot[h][:, ksl],
                                qrot[h][:, qc * c.QC + lo:(qc + 1) * c.QC],
                                start=True, stop=True)
                        return ps

                    acc = wp.tile([128, c.QC], F16, tag="acc", bufs=2,
                                  name=f"acc{qc}_{h}")
                    st_q = [pair_mm(m) for m in range(min(2, npair))]
                    for m in range(npair):
                        st_cur = st_q.pop(0)
                        if m + 2 < npair:
                            st_q.append(pair_mm(m + 2))
                        est = wp.tile([128, 2, c.QC], F16, tag="est", bufs=6)
                        if 2 * m + 1 - qc * c.kpq > 0:
                            # pair touches the diagonal: per-tile exps over
                            # the live columns (same-or-less ACT time, and
                            # the dead PSUM region is never read)
                            for i in (0, 1):
                                j = 2 * m + i - qc * c.kpq
                                lo = 128 * j if j > 0 else 0
                                nc.scalar.activation(est[:, i, lo:],
                                                     st_cur[:, i, lo:], EXP,
                                                     scale=scale_s)
                        else:
                            nc.scalar.activation(est[:], st_cur[:], EXP,
                                                 scale=scale_s)
                        for i in (0, 1):
                            kt = 2 * m + i
                            j = kt - qc * c.kpq
                            lo = 128 * j if j > 0 else 0
                            if j >= 0:
                                # mixed strip of the diagonal tile
                                nc.vector.tensor_tensor(
                                    est[:, i, 128 * j:128 * j + 128],
                                    est[:, i, 128 * j:128 * j + 128],
                                    tri[:], MUL)
                            nc.tensor.matmul(ctx_ps[:, lo:], v_sb[:, kt, hsl],
                                             est[:, i, lo:],
                                             start=(kt == 0), stop=False)
                            # elementwise est accumulation on the DVE;
                            # values stay O(30) so fp16 is safe
                            if kt == 0:
                                nc.vector.tensor_copy(acc[:], est[:, 0, :])
                            else:
                                nc.vector.tensor_tensor(
                                    acc[:, lo:], acc[:, lo:], est[:, i, lo:],
                                    ADD)
                        filler(1)
                    # adapter attention (no rope on adapter k, 1/sqrt(H))
                    ast_ps = psp.tile([c.L, c.QC], F32, tag="accps", bufs=2,
                                      name=f"ast{qc}_{h}")
                    nc.tensor.matmul(ast_ps[:], akt[:, h, :], qrot[h][:, sl],
                                     start=True, stop=True)
                    aest = wp.tile([c.L, c.QC], BF16, tag="aest", bufs=2,
                                   name=f"aest{qc}_{h}")
                    nc.scalar.activation(aest[:], ast_ps[:], EXP, scale=scale_a)
                    # per-head softmax denominators, one shared PSUM bank:
                    # rows 64h (main) and 64h+32 (adapter) -- matmul outputs
                    # must start at a 32-aligned partition
                    if sums_box[0] is None:
                        sums_box[0] = psp.tile([97, c.QC], F32, tag="accps",
                                               bufs=2, name=f"sums{qc}")
                    sums = sums_box[0]
                    nc.tensor.matmul(sums[64 * h:64 * h + 1, :],
                                     ones_t[:, 0:1], acc[:],
                                     start=True, stop=True,
                                     tile_position=(0, 64 * h))
                    nc.tensor.matmul(sums[64 * h + 32:64 * h + 33, :],
                                     ones_t[0:c.L, 0:1], aest[:],
                                     start=True, stop=True,
                                     tile_position=(0, 64 * h + 32))
                    # ---- early denominator readback (DVE): frees the sums
                    # bank and shortens the combine critical path
                    r1 = wp.tile([1, c.QC], F16, tag="r1", bufs=4,
                                 name=f"r1_{qc}_{h}")
                    ra32 = wp.tile([1, c.QC], F32, tag="ra32", bufs=2,
                                   name=f"ra32_{qc}_{h}")
                    f2 = wp.tile([1, c.QC], F16, tag="f2", bufs=2,
                                 name=f"f2_{qc}_{h}")
                    # adapter sums reach ~6e5, so 1/asum is fp16-SUBNORMAL;
                    # that reciprocal must stay fp32 (f2 = sum/asum itself
                    # is fp16-safe). 1/sum is ~1e-4..1 -> fp16 fine.
                    with nc.allow_low_precision(reason="1/softmax-sum fp16"):
                        nc.vector.reciprocal(r1[:], sums[64 * h:64 * h + 1])
                        nc.vector.reciprocal(ra32[:],
                                             sums[64 * h + 32:64 * h + 33])
                        nc.vector.tensor_tensor(f2[:], sums[64 * h:64 * h + 1],
                                                ra32[:], MUL)
                    state[h] = (ctx_ps, aest, r1, f2)

                def combine_head(qc, h, state, ct_tiles):
                    # combine: ct = (ctx + actx_g*sum/asum)/sum  (gate is
                    # pre-folded into av via the host-scaled aptv)
                    hsl = slice(h * 128, (h + 1) * 128)
                    ctx_ps, aest, r1, f2 = state[h]
                    # [1,QC] -> [L,QC] and [1,QC] -> [128,QC] broadcasts on
                    # the idle GPSIMD engine (PE/ACT are the scarce engines);
                    # sources sit at partition 0 as required
                    f10 = wp.tile([c.L, c.QC], F16, tag="f10", bufs=2,
                                  name=f"f10_{qc}_{h}")
                    nc.gpsimd.partition_broadcast(f10[:], f2[:])
                    aest2 = wp.tile([c.L, c.QC], F16, tag="aest2", bufs=2,
                                    name=f"aest2_{qc}_{h}")
                    nc.vector.tensor_tensor(aest2[:], aest[:], f10[:], MUL)
                    nc.tensor.matmul(ctx_ps[:], av_sb[:, hsl], aest2[:],
                                     start=False, stop=True)
                    rcb = wp.tile([128, c.QC], F16, tag="rcb", bufs=2,
                                  name=f"rcbs{qc}_{h}")
                    nc.gpsimd.partition_broadcast(rcb[:], r1[:])
                    ct = wp.tile([128, c.QC], F16, tag="ct", bufs=6,
                                 name=f"ct{qc}_{h}")
                    nc.vector.tensor_tensor(ct[:], ctx_ps[:], rcb[:], MUL)
                    ct_tiles[h] = ct

                def out_proj_units(qc, ct_tiles, copy_split=None):
                    # out_pT[do, q] += wot[:, h, do].T @ ct[h]  (local heads
                    # only; cross-core reduction happens on the host). One
                    # emit-closure per dt tile; interleaved into the next
                    # chunk's score loop as PE filler.
                    sl = slice(qc * c.QC, (qc + 1) * c.QC)

                    def unit(dt):
                        def emit():
                            dsl = slice(dt * 128, (dt + 1) * 128)
                            o_ps = psp.tile([128, c.QC], F32, tag="accps",
                                            bufs=2, name=f"o_ps{qc}_{dt}")
                            for h in range(c.hpc):
                                nc.tensor.matmul(o_ps[:], wot[:, h, dsl],
                                                 ct_tiles[h][:],
                                                 start=(h == 0),
                                                 stop=(h == c.hpc - 1))
                            o_sb = wp.tile([128, c.QC], F16, tag="osb", bufs=6,
                                           name=f"o_sb{qc}_{dt}")
                            if copy_split is None:
                                on_dve = dt % 2 == 1
                            else:
                                on_dve = dt < copy_split
                            if on_dve:
                                nc.vector.tensor_copy(o_sb[:], o_ps[:])
                            else:
                                nc.scalar.copy(o_sb[:], o_ps[:])
                            nc.sync.dma_start(out_d[dsl, sl], o_sb[:])
                        return emit
                    return [unit(dt) for dt in range(c.nd)]

                def out_proj_tail(qc, ct_tiles):
                    # final out-proj: 4 PSUM slots (accps + big pools) and a
                    # 2-ahead first-head window so the PE never waits on the
                    # PSUM-evacuation copies. Output DMA batched 4 dt-tiles
                    # per transfer on the SP ring (one issue slot, line-rate)
                    sl = slice(qc * c.QC, (qc + 1) * c.QC)
                    out_r = out_d.rearrange("(t p) s -> p t s", p=128)
                    o_ps = {}
                    grp = 4
                    o_sb4 = None

                    def open_dt(dt):
                        tag = "big" if dt % 2 == 0 else "accps"
                        ps = psp.tile([128, c.QC], F32, tag=tag, bufs=2,
                                      name=f"o_ps{qc}_{dt}")
                        dsl = slice(dt * 128, (dt + 1) * 128)
                        nc.tensor.matmul(ps[:], wot[:, 0, dsl], ct_tiles[0][:],
                                         start=True, stop=(c.hpc == 1))
                        o_ps[dt] = ps

                    # group bounds: big batches first, tiny last ones so
                    # the final transfer lands right after the final copy
                    cuts = list(range(grp, c.nd, grp)) + [c.nd]
                    gstart = 0
                    for dt in range(min(2, c.nd)):
                        open_dt(dt)
                    for dt in range(c.nd):
                        ps = o_ps.pop(dt)
                        dsl = slice(dt * 128, (dt + 1) * 128)
                        for h in range(1, c.hpc):
                            nc.tensor.matmul(ps[:], wot[:, h, dsl],
                                             ct_tiles[h][:],
                                             start=False, stop=(h == c.hpc - 1))
                        if dt + 2 < c.nd:
                            open_dt(dt + 2)
                        gend = min(cc for cc in cuts if cc > dt)
                        if dt == gstart:
                            o_sb4 = wp.tile([128, gend - gstart, c.QC], F16,
                                            tag="osb4", bufs=2,
                                            name=f"o_sb4_{qc}_{dt}")
                        if dt % 2 == 1:
                            nc.vector.tensor_copy(o_sb4[:, dt - gstart, :], ps[:])
                        else:
                            nc.scalar.copy(o_sb4[:, dt - gstart, :], ps[:])
                        if dt == gend - 1:
                            nc.sync.dma_start(out_r[:, gstart:gend, sl],
                                              o_sb4[:])
                            gstart = gend

                # ---------- fused pipeline ----------
                proj_chunk(0)
                adapter_kv()
                pending = []

                def filler(n):
                    for _ in range(n):
                        if pending:
                            pending.pop(0)()

                for qc in range(c.ns):
                    last = qc + 1 == c.ns
                    state, ct_tiles = {}, {}
                    sums_box = [None]
                    for h in range(c.hpc):
                        scores_head(qc, h, state, filler, sums_box)
                        if last and h + 1 < c.hpc:
                            # hide head h's combine chain under head h+1's
                            # k-loop (drained via the filler)
                            hh = h

                            def comb():
                                combine_head(qc, hh, state, ct_tiles)
                            pending.insert(0, comb)
                    while pending:
                        pending.pop(0)()
                    if not last:
                        proj_chunk(qc + 1)
                        for h in range(c.hpc):
                            combine_head(qc, h, state, ct_tiles)
                        units = out_proj_units(
                            qc, ct_tiles,
                            copy_split=(c.nd // 2 if qc + 2 == c.ns else None))
                        pending.extend(units)
                    else:
                        if c.hpc == 1:
                            combine_head(qc, 0, state, ct_tiles)
                        else:
                            combine_head(qc, c.hpc - 1, state, ct_tiles)
                        out_proj_tail(qc, ct_tiles)

    nc.compile()
    return nc


def make_in_maps(cfg, hidden_states, Wq, Wk, Wv, Wo, adaption_prompt,
                 adaption_gate, position_ids):
    """Host-side sharding: slice/transpose/cast per core + RoPE tables."""
    c = cfg
    x = np.asarray(hidden_states, np.float32)[0]          # [s, d]
    xt = np.ascontiguousarray(x.T).astype(NP_F16)         # [d, s]
    ap = np.asarray(adaption_prompt, np.float32)[0]       # [L, d]
    apt = np.ascontiguousarray(ap.T).astype(NP_F16)       # [d, L]
    gate = float(np.asarray(adaption_gate).reshape(-1)[0])
    aptv = np.ascontiguousarray(gate * ap.T).astype(NP_F16)
    # RoPE tables in the [hd, s] transposed layout; sin pre-signed.
    pos = np.asarray(position_ids).reshape(-1).astype(np.float64)  # [s]
    inv = 1.0 / (c.rope_base ** (np.arange(0, c.hd, 2, dtype=np.float64) / c.hd))
    f = inv[:, None] * pos[None, :]                       # [hd/2, s]
    cos_t = np.concatenate([np.cos(f), np.cos(f)], axis=0).astype(NP_F16)
    sv = np.sin(f)
    # halves swapped: rows 0:64 multiply q[0:64] (+sin, lands in dst[64:]),
    # rows 64:128 multiply q[64:128] (-sin, lands in dst[0:64])
    sin_t = np.concatenate([sv, -sv], axis=0).astype(NP_F16)
    in_maps = []
    for i in range(c.n_cores):
        rs = slice(i * c.dh, (i + 1) * c.dh)
        wq_t = np.asarray(Wq, np.float32)[rs, :].T.astype(NP_F16)   # [d, dh]
        wk_t = np.asarray(Wk, np.float32)[rs, :].T.astype(NP_F16)
        wqk = np.ascontiguousarray(np.stack([wq_t, wk_t], axis=1))  # [d, 2, dh]
        in_maps.append({
            "xt": xt,
            "wqk": wqk,
            "wvt": np.ascontiguousarray(np.asarray(Wv, np.float32)[rs, :].T).astype(NP_F16),
            "wot": np.ascontiguousarray(np.asarray(Wo, np.float32)[:, rs].T).astype(NP_F16),
            "apt": apt,
            "aptv": aptv,
            "cost": cos_t,
            "sint": sin_t,
        })
    return in_maps


def assemble_output(cfg, results):
    acc = np.zeros((cfg.d, cfg.s), np.float32)
    for r in results:
        acc += np.asarray(r["out"], np.float32)           # per-core partial [d, s]
    return np.ascontiguousarray(acc.T)[None]              # [1, s, d]


_NC_CACHE = {}


def run(inputs, cfg=None, trace=False):
    cfg = cfg or Cfg()
    key = (cfg.s, cfg.d, cfg.L, cfg.n_cores, cfg.n_heads)
    if key not in _NC_CACHE:
        _NC_CACHE[key] = build(cfg)
    nc = _NC_CACHE[key]
    in_maps = make_in_maps(cfg, **inputs)
    res = run_bass_kernel_spmd(nc, in_maps, core_ids=list(range(cfg.n_cores)),
                               trace=trace)
    out = assemble_output(cfg, res.results)
    return out, res


def kernel(**inputs) -> np.ndarray:
    out, _ = run(inputs)
    return out.astype(np.float32)


# revision 16
# speedup vs baseline: 1.0495x; 1.0495x over previous
"""AdaptedAttention (llama + adaption-prompt) on 8 TRN2 NeuronCores.

Sharding: tensor-parallel over heads (2 heads/core), zero device collectives.
Per core (everything on-chip fp16; PSUM accumulation fp32):
  - qT/kT/V projections for its 2 heads from fp16, pre-transposed X
    (all matmuls contract over d, so X lives on-chip as X.T [d part, s free]).
    The q/k projection t-loop is OUTERMOST (4 open PSUM accumulation groups:
    q+k x 2 heads) so chunk-0's PE consumption order matches the startup DMA
    arrival order t-slice by t-slice -- PE starts ~1 us after launch and
    streams at the DMA rate instead of stalling on late weight slices.
  - RoPE in the [hd, s] layout with HOST-precomputed fp16 cos/sin tables
    (sin pre-signed AND half-swapped so each DVE TensorTensor's two SBUF
    inputs share a base partition -- a HW requirement)
  - attention as S^T = K @ Q^T tiles ([k part, q free]) so softmax needs no
    transposes. Score k-tiles are computed in PAIRS into one [128, 2, QC]
    fp32 PSUM tile (2 banks) so ONE ACT exp covers 1024 columns -- the ACT
    per-op init (~185 ns) amortizes over 2 tiles, which matters because the
    k-loop is ACT-throughput-bound. Causal handling: skip k-tiles above the
    diagonal, col-restricted ctx/acc reads on diagonal tiles (the below-diag
    exp garbage is never read), and one [128,128] triangular fp16 mask for
    the 128-wide mixed strip of each diagonal tile. Row sums: est tiles
    accumulate elementwise on the DVE (fp16 2x), then ONE ones-matmul
    partition-reduce per head into a shared [97, QC] sums bank (rows 0/32/
    64/96: main/adapter x head -- matmul outputs need 32-aligned bases).
  - softmax denominators are read back (DVE reciprocals) IMMEDIATELY after
    each head's k-loop: the sums bank shares a pool with out-proj PSUM
    tiles, and freeing it early unblocks the next chunk's allocations.
  - adapter path (L=10) folded into the main ctx PSUM accumulation:
    ct = (ctx + actx_g * sum/asum) / sum, gate pre-folded into the
    host-scaled adapter-V copy (aptv). 1/asum stays fp32 (asum ~ 6e5 makes
    it fp16-subnormal). The [1,QC] -> [L,QC] and [1,QC] -> [128,QC]
    broadcasts run on the otherwise-idle GPSIMD engine (partition_broadcast)
    instead of burning PE matmuls + ACT copies.
  - output projection uses only the core's OWN 2-head ct against its
    256-column slice of Wo, producing fp16 partials [d, s]; the cross-core
    sum happens on the host as the unshard step. out_proj(qc) is NOT
    emitted right after chunk qc: it is deferred and interleaved, one
    dt-tile at a time, into chunk qc+1's k-loop emission, so the PE has
    filler work during the ACT-bound stretches of the score loop (the PE
    engine queue is FIFO -- anything emitted later cannot jump ahead).
    In the LAST chunk, head 0's combine chain is likewise deferred into
    head 1's k-loop (hiding its DVE/Pool latency under PE score work), and
    the final out-proj ping-pongs over FOUR PSUM slots (2 pools) with a
    2-ahead first-head window so the tail runs at PE rate, not at
    PSUM-evacuation-copy latency.
PSUM banks (8): big x2 (score-pairs / merged q+k proj / v proj / adapter
projections / tail out-proj; [128,2,QC] fp32 slots = 2 banks each), ctx x2,
accps x2 (sums / adapter scores / out-proj accumulators).
Host side: weight slicing/transposes/casts, RoPE tables from position_ids,
partial-sum + transpose.
"""

import math
import numpy as np

import concourse.bass as bass
import concourse.bacc as bacc
import concourse.mybir as mybir
import concourse.tile as tile
from concourse.bass_utils import run_bass_kernel_spmd

F16 = mybir.dt.float16
BF16 = mybir.dt.bfloat16
F32 = mybir.dt.float32
NP_F16 = mybir.dt.np(F16)
NP_BF16 = mybir.dt.np(BF16)


class Cfg:
    def __init__(self, s=2048, d=2048, L=10, n_cores=8, n_heads=16, rope_base=10000.0):
        self.s, self.d, self.L = s, d, L
        self.n_cores = n_cores
        self.n_heads = n_heads
        self.rope_base = rope_base
        self.hd = 128                      # head dim (fixed)
        self.hpc = n_heads // n_cores      # heads per core
        self.dh = self.hpc * self.hd       # local head-dim cols per core
        self.nd = d // 128                 # contraction chunks
        self.QC = 512                      # q-chunk width
        self.ns = s // self.QC             # q-chunks
        self.nst = s // 128                # s tiles (k tiles)
        self.kpq = self.QC // 128          # k-tiles straddling one q-chunk diag
        assert self.hpc * n_cores == n_heads and d % 128 == 0 and s % self.QC == 0
        assert self.kpq % 2 == 0


def build(cfg: Cfg, nrep: int = 1):
    """Build the per-core SPMD graph. Returns compiled nc.
    nrep>1 repeats the whole pipeline (for marginal-time HW measurement)."""
    c = cfg
    nc = bacc.Bacc(None, target_bir_lowering=False, num_devices=c.n_cores)

    # ---------------- external I/O (per-core shards) ----------------
    xt_d = nc.dram_tensor("xt", [c.d, c.s], F16, kind="ExternalInput")
    wqk_d = nc.dram_tensor("wqk", [c.d, 2, c.dh], F16, kind="ExternalInput")
    wvt_d = nc.dram_tensor("wvt", [c.d, c.dh], F16, kind="ExternalInput")
    wot_d = nc.dram_tensor("wot", [c.dh, c.d], F16, kind="ExternalInput")
    apt_d = nc.dram_tensor("apt", [c.d, c.L], F16, kind="ExternalInput")
    aptv_d = nc.dram_tensor("aptv", [c.d, c.L], F16, kind="ExternalInput")
    cos_d = nc.dram_tensor("cost", [128, c.s], F16, kind="ExternalInput")
    sin_d = nc.dram_tensor("sint", [128, c.s], F16, kind="ExternalInput")
    out_d = nc.dram_tensor("out", [c.d, c.s], F16, kind="ExternalOutput")

    # single [128,128] lower-triangular mask: tri[k, q] = 1 if k <= q.
    # Diagonal k-tile j of a chunk only mixes masked/unmasked inside a
    # 128-wide column strip; columns below it are handled by col-restricted
    # reads, columns above are fully unmasked.
    kk = np.arange(128)[:, None]
    qq = np.arange(128)[None, :]
    tri_np = (kk <= qq).astype(NP_F16)  # [128, 128]
    tri_d = nc.inline_tensor(tri_np, name="tri")

    scale_s = 1.0 / math.sqrt(c.hd)        # main attention scale
    scale_a = 1.0 / math.sqrt(c.n_heads)   # adapter scale (faithful to ref)

    EXP = mybir.ActivationFunctionType.Exp
    ADD = mybir.AluOpType.add
    MUL = mybir.AluOpType.mult

    with tile.TileContext(nc) as tc:
        with (
            tc.tile_pool(name="bigsb", bufs=1) as bigp,
            tc.tile_pool(name="persist", bufs=1) as pp,
            tc.tile_pool(name="work", bufs=3) as wp,
            tc.tile_pool(name="psum", bufs=1, space="PSUM") as psp,
        ):
            if nrep == 0:
                # timing baseline: touch every input (the terminal only ships
                # buffers the NEFF references) but do ~zero device work
                z = pp.tile([1, 128], F16, tag="z")
                for i, t in enumerate((xt_d, wqk_d, wvt_d, wot_d,
                                       apt_d, aptv_d, cos_d, sin_d)):
                    nc.sync.dma_start(z[0:1, 8 * i:8 * i + 8], t[0:1, 0:8])
                zo = pp.tile([1, 64], F16, tag="zo")
                nc.gpsimd.memset(zo[:], 0.0)
                nc.sync.dma_start(out_d[0:1, 0:64], zo[:])
            for _rep in range(nrep):
                # ---------- loads (q/k weights + chunk0, t-sliced, first) ----
                # q and k weights interleaved per t-slice so one DMA
                # stream delivers them in exactly the consumption order
                wqk = pp.tile([128, c.nd, 2, c.dh], F16, tag="wqk")
                wqk_r = wqk_d.rearrange("(t p) two m -> p t two m", p=128)
                xt = bigp.tile([128, c.nd, c.s], F16, tag="big")
                xt_r = xt_d.rearrange("(t p) s -> p t s", p=128)
                # t-sliced startup loads, arrival order == the t-interleaved
                # consumption order of proj_chunk(0): wq/wk ride the SP HWDGE
                # ring while chunk-0 x slices ride the otherwise-idle ACT ring
                bounds = [b for b in (0, 1, 3, 6, 10, 13, c.nd) if b <= c.nd]
                bounds = sorted(set(bounds + [c.nd]))
                cos_t = pp.tile([128, c.s], F16, tag="cos")
                sin_t = pp.tile([128, c.s], F16, tag="sin")
                for i in range(len(bounds) - 1):
                    ts = slice(bounds[i], bounds[i + 1])
                    nc.sync.dma_start(wqk[:, ts, :, :], wqk_r[:, ts, :, :])
                    nc.scalar.dma_start(xt[:, ts, 0:c.QC], xt_r[:, ts, 0:c.QC])
                nc.sync.dma_start(cos_t[:, 0:c.QC], cos_d[:, 0:c.QC])
                nc.sync.dma_start(sin_t[:, 0:c.QC], sin_d[:, 0:c.QC])
                # dummy exp: pulls the ACT function-table load into the
                # startup DMA window instead of stalling the first real copy
                warm = wp.tile([1, 2], F32, tag="warm", bufs=1)
                nc.vector.memset(warm[:], 0.0)
                nc.scalar.activation(warm[:], warm[:], EXP, scale=1.0)
                tri = pp.tile([128, 128], F16, tag="tri")
                nc.sync.dma_start(tri[:], tri_d[:])
                # load order follows first-use time: wvt feeds the chunk-0
                # v-projection (~19 us) BEFORE adapter_kv needs apt (~21 us)
                wvt = pp.tile([128, c.nd, c.dh], F16, tag="wvt")
                nc.sync.dma_start(wvt[:], wvt_d.rearrange("(t p) m -> p t m", p=128))
                apt = pp.tile([128, c.nd, c.L], F16, tag="apt")
                nc.sync.dma_start(apt[:], apt_d.rearrange("(t p) m -> p t m", p=128))
                aptv = pp.tile([128, c.nd, c.L], F16, tag="aptv")
                nc.sync.dma_start(aptv[:], aptv_d.rearrange("(t p) m -> p t m", p=128))
                if c.s > c.QC:
                    sl = slice(c.QC, c.s)
                    nc.sync.dma_start(cos_t[:, sl], cos_d[:, sl])
                    nc.sync.dma_start(sin_t[:, sl], sin_d[:, sl])
                for qc in range(1, c.ns):
                    sl = slice(qc * c.QC, (qc + 1) * c.QC)
                    nc.sync.dma_start(xt[:, :, sl], xt_r[:, :, sl])
                wot = pp.tile([128, c.hpc, c.d], F16, tag="wot")
                nc.sync.dma_start(wot[:], wot_d.rearrange("(t p) m -> p t m", p=128))
                # all-ones: column [:, 0:1] is the row-sum lhsT
                ones_t = pp.tile([128, 128], F16, tag="ones_t")
                nc.gpsimd.memset(ones_t[:], 1.0)

                # ---------- persistent intermediates ----------
                qrot = [pp.tile([128, c.s], F16, tag=f"qrot{h}", name=f"qrot{h}")
                        for h in range(c.hpc)]
                krot = [pp.tile([128, c.s], F16, tag=f"krot{h}", name=f"krot{h}")
                        for h in range(c.hpc)]
                v_sb = pp.tile([128, c.nst, c.dh], F16, tag="v")
                akt = pp.tile([128, c.hpc, c.L], F16, tag="akt")
                av_sb = pp.tile([c.L, c.dh], F16, tag="av")

                def rope_copy(src_ps, on_act):
                    # PSUM -> SBUF evacuation, FIRST so the projection bank
                    # frees after one short copy instead of a full TT chain;
                    # half the copies ride the otherwise-idle ACT engine
                    qf = wp.tile([128, c.QC], F16, tag="qf", bufs=4)
                    if on_act:
                        nc.scalar.copy(qf[:], src_ps[:])
                    else:
                        nc.vector.tensor_copy(qf[:], src_ps[:])
                    return qf

                def rope_tt(dst, qf, sl):
                    # dst[0:64]   = src[0:64]*cos[0:64] - src[64:]*sin[0:64]
                    # dst[64:128] = src[64:]*cos[64:]   + src[0:64]*sin[64:]
                    # sin_t is pre-signed on host: rows 0:64 hold -sin, and
                    # halves are swapped so each TT's two SBUF inputs share a
                    # base partition (HW requirement)
                    t2 = wp.tile([128, c.QC], F16, tag="tmp", bufs=6)
                    nc.vector.tensor_tensor(t2[0:64], qf[64:128],
                                            sin_t[64:128, sl], MUL)
                    nc.vector.tensor_tensor(t2[64:128], qf[0:64],
                                            sin_t[0:64, sl], MUL)
                    t1 = wp.tile([128, c.QC], F16, tag="tmp", bufs=6)
                    nc.vector.tensor_tensor(t1[:], qf[:], cos_t[:, sl], MUL)
                    nc.vector.tensor_tensor(dst, t1[:], t2[:], ADD)

                def proj_chunk(qc):
                    sl = slice(qc * c.QC, (qc + 1) * c.QC)
                    if qc == 0:
                        # t OUTERMOST (4 open accumulation groups: q+k x 2
                        # heads) so chunk-0 consumption matches the startup
                        # DMA arrival order t-slice by t-slice
                        qk_ps = [psp.tile([128, 2, c.QC], F32, tag="big",
                                          bufs=2, name=f"qk{qc}_{h}")
                                 for h in range(c.hpc)]
                        for t in range(c.nd):
                            for h in range(c.hpc):
                                hsl = slice(h * 128, (h + 1) * 128)
                                nc.tensor.matmul(qk_ps[h][:, 0, :],
                                                 wqk[:, t, 0, hsl], xt[:, t, sl],
                                                 start=(t == 0),
                                                 stop=(t == c.nd - 1))
                                nc.tensor.matmul(qk_ps[h][:, 1, :],
                                                 wqk[:, t, 1, hsl], xt[:, t, sl],
                                                 start=(t == 0),
                                                 stop=(t == c.nd - 1))
                        qfs = []
                        for h in range(c.hpc):
                            qfs.append(rope_copy(qk_ps[h][:, 0, :], False))
                            qfs.append(rope_copy(qk_ps[h][:, 1, :], True))
                        for h in range(c.hpc):
                            rope_tt(qrot[h][:, sl], qfs[2 * h], sl)
                            rope_tt(krot[h][:, sl], qfs[2 * h + 1], sl)
                    else:
                        for h in range(c.hpc):
                            hsl = slice(h * 128, (h + 1) * 128)
                            qk_ps = psp.tile([128, 2, c.QC], F32, tag="big",
                                             bufs=2, name=f"qk{qc}_{h}")
                            for t in range(c.nd):
                                nc.tensor.matmul(qk_ps[:, 0, :], wqk[:, t, 0, hsl],
                                                 xt[:, t, sl], start=(t == 0),
                                                 stop=(t == c.nd - 1))
                            for t in range(c.nd):
                                nc.tensor.matmul(qk_ps[:, 1, :], wqk[:, t, 1, hsl],
                                                 xt[:, t, sl], start=(t == 0),
                                                 stop=(t == c.nd - 1))
                            qf_q = rope_copy(qk_ps[:, 0, :], False)
                            qf_k = rope_copy(qk_ps[:, 1, :], True)
                            rope_tt(qrot[h][:, sl], qf_q, sl)
                            rope_tt(krot[h][:, sl], qf_k, sl)
                    for st in range(c.kpq):
                        gst = qc * c.kpq + st
                        ssl = slice(gst * 128, (gst + 1) * 128)
                        # accps pool: free during every proj window, so the
                        # v matmuls never wait on the rope copies that hold
                        # the big slots
                        v_ps = psp.tile([128, c.dh], F32, tag="accps", bufs=2,
                                        name=f"v{qc}_{st}")
                        for t in range(c.nd):
                            nc.tensor.matmul(v_ps[:], xt[:, t, ssl], wvt[:, t, :],
                                             start=(t == 0), stop=(t == c.nd - 1))
                        nc.scalar.copy(v_sb[:, gst, :], v_ps[:])

                def adapter_kv():
                    for h in range(c.hpc):
                        hsl = slice(h * 128, (h + 1) * 128)
                        a_ps = psp.tile([128, c.L], F32, tag="big", bufs=2)
                        for t in range(c.nd):
                            nc.tensor.matmul(a_ps[:], wqk[:, t, 1, hsl], apt[:, t, :],
                                             start=(t == 0), stop=(t == c.nd - 1))
                        nc.scalar.copy(akt[:, h, :], a_ps[:])
                    av_ps = psp.tile([c.L, c.dh], F32, tag="big", bufs=2)
                    for t in range(c.nd):
                        nc.tensor.matmul(av_ps[:], aptv[:, t, :], wvt[:, t, :],
                                         start=(t == 0), stop=(t == c.nd - 1))
                    nc.scalar.copy(av_sb[:], av_ps[:])

                def scores_head(qc, h, state, filler, sums_box):
                    sl = slice(qc * c.QC, (qc + 1) * c.QC)
                    nkt = qc * c.kpq + c.kpq  # causal: k-tiles 0..nkt-1
                    npair = nkt // 2
                    hsl = slice(h * 128, (h + 1) * 128)
                    ctx_ps = psp.tile([128, c.QC], F32, tag="ctx", bufs=2,
                                      name=f"ctx{qc}_{h}")

                    def pair_mm(m):
                        # two k-tiles of S^T into one 2-bank fp32 tile so
                        # the exp below covers 1024 columns in one op; on
                        # diagonal tiles only the live columns are computed
                        ps = psp.tile([128, 2, c.QC], F32, tag="big", bufs=2,
                                      name=f"st{qc}_{h}_{m}")
                        for i in (0, 1):
                            kt = 2 * m + i
                            j = kt - qc * c.kpq
                            lo = 128 * j if j > 0 else 0
                            ksl = slice(kt * 128, (kt + 1) * 128)
                            nc.tensor.matmul(
                                ps[:, i, lo:], krot[h][:, ksl],
                                qrot[h][:, qc * c.QC + lo:(qc + 1) * c.QC],
                                start=True, stop=True)
                        return ps

                    acc = wp.tile([128, c.QC], F16, tag="acc", bufs=2,
                                  name=f"acc{qc}_{h}")
                    st_q = [pair_mm(m) for m in range(min(2, npair))]
                    for m in range(npair):
                        st_cur = st_q.pop(0)
                        if m + 2 < npair:
                            st_q.append(pair_mm(m + 2))
                        est = wp.tile([128, 2, c.QC], F16, tag="est", bufs=6)
                        if 2 * m + 1 - qc * c.kpq > 0:
                            # pair touches the diagonal: per-tile exps over
                            # the live columns (same-or-less ACT time, and
                            # the dead PSUM region is never read)
                            for i in (0, 1):
                                j = 2 * m + i - qc * c.kpq
                                lo = 128 * j if j > 0 else 0
                                nc.scalar.activation(est[:, i, lo:],
                                                     st_cur[:, i, lo:], EXP,
                                                     scale=scale_s)
                        else:
                            nc.scalar.activation(est[:], st_cur[:], EXP,
                                                 scale=scale_s)
                        for i in (0, 1):
                            kt = 2 * m + i
                            j = kt - qc * c.kpq
                            lo = 128 * j if j > 0 else 0
                            if j >= 0:
                                # mixed strip of the diagonal tile
                                nc.vector.tensor_tensor(
                                    est[:, i, 128 * j:128 * j + 128],
                                    est[:, i, 128 * j:128 * j + 128],
                                    tri[:], MUL)
                            nc.tensor.matmul(ctx_ps[:, lo:], v_sb[:, kt, hsl],
                                             est[:, i, lo:],
                                             start=(kt == 0), stop=False)
                            # elementwise est accumulation on the DVE;
                            # values stay O(30) so fp16 is safe
                            if kt == 0:
                                nc.vector.tensor_copy(acc[:], est[:, 0, :])
                            else:
                                nc.vector.tensor_tensor(
                                    acc[:, lo:], acc[:, lo:], est[:, i, lo:],
                                    ADD)
                        filler(1)
                    # adapter attention (no rope on adapter k, 1/sqrt(H))
                    ast_ps = psp.tile([c.L, c.QC], F32, tag="accps", bufs=2,
                                      name=f"ast{qc}_{h}")
                    nc.tensor.matmul(ast_ps[:], akt[:, h, :], qrot[h][:, sl],
                                     start=True, stop=True)
                    aest = wp.tile([c.L, c.QC], BF16, tag="aest", bufs=2,
                                   name=f"aest{qc}_{h}")
                    nc.scalar.activation(aest[:], ast_ps[:], EXP, scale=scale_a)
                    # per-head softmax denominators, one shared PSUM bank:
                    # rows 64h (main) and 64h+32 (adapter) -- matmul outputs
                    # must start at a 32-aligned partition
                    if sums_box[0] is None:
                        sums_box[0] = psp.tile([97, c.QC], F32, tag="accps",
                                               bufs=2, name=f"sums{qc}")
                    sums = sums_box[0]
                    nc.tensor.matmul(sums[64 * h:64 * h + 1, :],
                                     ones_t[:, 0:1], acc[:],
                                     start=True, stop=True,
                                     tile_position=(0, 64 * h))
                    nc.tensor.matmul(sums[64 * h + 32:64 * h + 33, :],
                                     ones_t[0:c.L, 0:1], aest[:],
                                     start=True, stop=True,
                                     tile_position=(0, 64 * h + 32))
                    # ---- early denominator readback (DVE): frees the sums
                    # bank and shortens the combine critical path
                    r1 = wp.tile([1, c.QC], F16, tag="r1", bufs=4,
                                 name=f"r1_{qc}_{h}")
                    ra32 = wp.tile([1, c.QC], F32, tag="ra32", bufs=2,
                                   name=f"ra32_{qc}_{h}")
                    f2 = wp.tile([1, c.QC], F16, tag="f2", bufs=2,
                                 name=f"f2_{qc}_{h}")
                    # adapter sums reach ~6e5, so 1/asum is fp16-SUBNORMAL;
                    # that reciprocal must stay fp32 (f2 = sum/asum itself
                    # is fp16-safe). 1/sum is ~1e-4..1 -> fp16 fine.
                    with nc.allow_low_precision(reason="1/softmax-sum fp16"):
                        nc.vector.reciprocal(r1[:], sums[64 * h:64 * h + 1])
                        nc.vector.reciprocal(ra32[:],
                                             sums[64 * h + 32:64 * h + 33])
                        nc.vector.tensor_tensor(f2[:], sums[64 * h:64 * h + 1],
                                                ra32[:], MUL)
                    state[h] = (ctx_ps, aest, r1, f2)

                def combine_head(qc, h, state, ct_tiles):
                    # combine: ct = (ctx + actx_g*sum/asum)/sum  (gate is
                    # pre-folded into av via the host-scaled aptv). The
                    # [1,QC] -> [L,QC] / [1,QC] -> [128,QC] broadcasts are
                    # ones-matmuls (216 ns on PE); gpsimd partition_broadcast
                    # measured FAR slower on HW (software Q7 loop + DVE-port
                    # contention) despite the cost model liking it. PSUM for
                    # them comes from the "big" pool, whose slots recycle
                    # fast (score-pair exps), so the last chunk's deferred
                    # combine cannot deadlock on the sums bank.
                    hsl = slice(h * 128, (h + 1) * 128)
                    ctx_ps, aest, r1, f2 = state[h]
                    f10_ps = psp.tile([c.L, c.QC], F32, tag="big", bufs=2,
                                      name=f"f10_{qc}_{h}")
                    nc.tensor.matmul(f10_ps[:], ones_t[0:1, 0:c.L], f2[:],
                                     start=True, stop=True)
                    aest2 = wp.tile([c.L, c.QC], F16, tag="aest2", bufs=2,
                                    name=f"aest2_{qc}_{h}")
                    nc.vector.tensor_tensor(aest2[:], aest[:], f10_ps[:], MUL)
                    nc.tensor.matmul(ctx_ps[:], av_sb[:, hsl], aest2[:],
                                     start=False, stop=True)
                    rcb_ps = psp.tile([128, c.QC], F32, tag="big", bufs=2,
                                      name=f"rcb_{qc}_{h}")
                    nc.tensor.matmul(rcb_ps[:], ones_t[0:1, :], r1[:],
                                     start=True, stop=True)
                    rcb = wp.tile([128, c.QC], F16, tag="rcb", bufs=2,
                                  name=f"rcbs{qc}_{h}")
                    nc.scalar.copy(rcb[:], rcb_ps[:])
                    ct = wp.tile([128, c.QC], F16, tag="ct", bufs=6,
                                 name=f"ct{qc}_{h}")
                    nc.vector.tensor_tensor(ct[:], ctx_ps[:], rcb[:], MUL)
                    ct_tiles[h] = ct

                def out_proj_units(qc, ct_tiles, copy_split=None):
                    # out_pT[do, q] += wot[:, h, do].T @ ct[h]  (local heads
                    # only; cross-core reduction happens on the host). One
                    # emit-closure per dt tile; interleaved into the next
                    # chunk's score loop as PE filler.
                    sl = slice(qc * c.QC, (qc + 1) * c.QC)

                    def unit(dt):
                        def emit():
                            dsl = slice(dt * 128, (dt + 1) * 128)
                            o_ps = psp.tile([128, c.QC], F32, tag="accps",
                                            bufs=2, name=f"o_ps{qc}_{dt}")
                            for h in range(c.hpc):
                                nc.tensor.matmul(o_ps[:], wot[:, h, dsl],
                                                 ct_tiles[h][:],
                                                 start=(h == 0),
                                                 stop=(h == c.hpc - 1))
                            o_sb = wp.tile([128, c.QC], F16, tag="osb", bufs=6,
                                           name=f"o_sb{qc}_{dt}")
                            if copy_split is None:
                                on_dve = dt % 2 == 1
                            else:
                                on_dve = dt < copy_split
                            if on_dve:
                                nc.vector.tensor_copy(o_sb[:], o_ps[:])
                            else:
                                nc.scalar.copy(o_sb[:], o_ps[:])
                            nc.sync.dma_start(out_d[dsl, sl], o_sb[:])
                        return emit
                    return [unit(dt) for dt in range(c.nd)]

                def out_proj_tail(qc, ct_tiles):
                    # final out-proj: 4 PSUM slots (accps + big pools) and a
                    # 2-ahead first-head window so the PE never waits on the
                    # PSUM-evacuation copies. Output DMA batched 4 dt-tiles
                    # per transfer on the SP ring (one issue slot, line-rate)
                    sl = slice(qc * c.QC, (qc + 1) * c.QC)
                    out_r = out_d.rearrange("(t p) s -> p t s", p=128)
                    o_ps = {}
                    grp = 4
                    o_sb4 = None

                    def open_dt(dt):
                        tag = "big" if dt % 2 == 0 else "accps"
                        ps = psp.tile([128, c.QC], F32, tag=tag, bufs=2,
                                      name=f"o_ps{qc}_{dt}")
                        dsl = slice(dt * 128, (dt + 1) * 128)
                        nc.tensor.matmul(ps[:], wot[:, 0, dsl], ct_tiles[0][:],
                                         start=True, stop=(c.hpc == 1))
                        o_ps[dt] = ps

                    # group bounds: big batches first, tiny last ones so
                    # the final transfer lands right after the final copy
                    cuts = list(range(grp, c.nd, grp)) + [c.nd]
                    gstart = 0
                    for dt in range(min(2, c.nd)):
                        open_dt(dt)
                    for dt in range(c.nd):
                        ps = o_ps.pop(dt)
                        dsl = slice(dt * 128, (dt + 1) * 128)
                        for h in range(1, c.hpc):
                            nc.tensor.matmul(ps[:], wot[:, h, dsl],
                                             ct_tiles[h][:],
                                             start=False, stop=(h == c.hpc - 1))
                        if dt + 2 < c.nd:
                            open_dt(dt + 2)
                        gend = min(cc for cc in cuts if cc > dt)
                        if dt == gstart:
                            o_sb4 = wp.tile([128, gend - gstart, c.QC], F16,
                                            tag="osb4", bufs=2,
                                            name=f"o_sb4_{qc}_{dt}")
                        if dt % 2 == 1:
                            nc.vector.tensor_copy(o_sb4[:, dt - gstart, :], ps[:])
                        else:
                            nc.scalar.copy(o_sb4[:, dt - gstart, :], ps[:])
                        if dt == gend - 1:
                            nc.sync.dma_start(out_r[:, gstart:gend, sl],
                                              o_sb4[:])
                            gstart = gend

                # ---------- fused pipeline ----------
                proj_chunk(0)
                adapter_kv()
                pending = []

                def filler(n):
                    for _ in range(n):
                        if pending:
                            pending.pop(0)()

                for qc in range(c.ns):
                    last = qc + 1 == c.ns
                    state, ct_tiles = {}, {}
                    sums_box = [None]
                    for h in range(c.hpc):
                        scores_head(qc, h, state, filler, sums_box)
                        if last and h + 1 < c.hpc:
                            # hide head h's combine chain under head h+1's
                            # k-loop (drained via the filler)
                            hh = h

                            def comb():
                                combine_head(qc, hh, state, ct_tiles)
                            pending.insert(0, comb)
                    while pending:
                        pending.pop(0)()
                    if not last:
                        proj_chunk(qc + 1)
                        for h in range(c.hpc):
                            combine_head(qc, h, state, ct_tiles)
                        units = out_proj_units(qc, ct_tiles)
                        pending.extend(units)
                    else:
                        if c.hpc == 1:
                            combine_head(qc, 0, state, ct_tiles)
                        else:
                            combine_head(qc, c.hpc - 1, state, ct_tiles)
                        out_proj_tail(qc, ct_tiles)

    nc.compile()
    return nc


def make_in_maps(cfg, hidden_states, Wq, Wk, Wv, Wo, adaption_prompt,
                 adaption_gate, position_ids):
    """Host-side sharding: slice/transpose/cast per core + RoPE tables."""
    c = cfg
    x = np.asarray(hidden_states, np.float32)[0]          # [s, d]
    xt = np.ascontiguousarray(x.T).astype(NP_F16)         # [d, s]
    ap = np.asarray(adaption_prompt, np.float32)[0]       # [L, d]
    apt = np.ascontiguousarray(ap.T).astype(NP_F16)       # [d, L]
    gate = float(np.asarray(adaption_gate).reshape(-1)[0])
    aptv = np.ascontiguousarray(gate * ap.T).astype(NP_F16)
    # RoPE tables in the [hd, s] transposed layout; sin pre-signed.
    pos = np.asarray(position_ids).reshape(-1).astype(np.float64)  # [s]
    inv = 1.0 / (c.rope_base ** (np.arange(0, c.hd, 2, dtype=np.float64) / c.hd))
    f = inv[:, None] * pos[None, :]                       # [hd/2, s]
    cos_t = np.concatenate([np.cos(f), np.cos(f)], axis=0).astype(NP_F16)
    sv = np.sin(f)
    # halves swapped: rows 0:64 multiply q[0:64] (+sin, lands in dst[64:]),
    # rows 64:128 multiply q[64:128] (-sin, lands in dst[0:64])
    sin_t = np.concatenate([sv, -sv], axis=0).astype(NP_F16)
    in_maps = []
    for i in range(c.n_cores):
        rs = slice(i * c.dh, (i + 1) * c.dh)
        wq_t = np.asarray(Wq, np.float32)[rs, :].T.astype(NP_F16)   # [d, dh]
        wk_t = np.asarray(Wk, np.float32)[rs, :].T.astype(NP_F16)
        wqk = np.ascontiguousarray(np.stack([wq_t, wk_t], axis=1))  # [d, 2, dh]
        in_maps.append({
            "xt": xt,
            "wqk": wqk,
            "wvt": np.ascontiguousarray(np.asarray(Wv, np.float32)[rs, :].T).astype(NP_F16),
            "wot": np.ascontiguousarray(np.asarray(Wo, np.float32)[:, rs].T).astype(NP_F16),
            "apt": apt,
            "aptv": aptv,
            "cost": cos_t,
            "sint": sin_t,
        })
    return in_maps


def assemble_output(cfg, results):
    acc = np.zeros((cfg.d, cfg.s), np.float32)
    for r in results:
        acc += np.asarray(r["out"], np.float32)           # per-core partial [d, s]
    return np.ascontiguousarray(acc.T)[None]              # [1, s, d]


_NC_CACHE = {}


def run(inputs, cfg=None, trace=False):
    cfg = cfg or Cfg()
    key = (cfg.s, cfg.d, cfg.L, cfg.n_cores, cfg.n_heads)
    if key not in _NC_CACHE:
        _NC_CACHE[key] = build(cfg)
    nc = _NC_CACHE[key]
    in_maps = make_in_maps(cfg, **inputs)
    res = run_bass_kernel_spmd(nc, in_maps, core_ids=list(range(cfg.n_cores)),
                               trace=trace)
    out = assemble_output(cfg, res.results)
    return out, res


def kernel(**inputs) -> np.ndarray:
    out, _ = run(inputs)
    return out.astype(np.float32)


# revision 21
# speedup vs baseline: 1.1115x; 1.0592x over previous
"""AdaptedAttention (llama + adaption-prompt) on 8 TRN2 NeuronCores.

Sharding: tensor-parallel over heads (2 heads/core), zero device collectives.
Per core (everything on-chip fp16; PSUM accumulation fp32):
  - qT/kT/V projections for its 2 heads from fp16, pre-transposed X
    (all matmuls contract over d, so X lives on-chip as X.T [d part, s free]).
    The q/k projection t-loop is OUTERMOST (4 open PSUM accumulation groups:
    q+k x 2 heads) so chunk-0's PE consumption order matches the startup DMA
    arrival order t-slice by t-slice -- PE starts ~1 us after launch and
    streams at the DMA rate instead of stalling on late weight slices.
  - RoPE in the [hd, s] layout with HOST-precomputed fp16 cos/sin tables
    (sin pre-signed AND half-swapped so each DVE TensorTensor's two SBUF
    inputs share a base partition -- a HW requirement)
  - attention as S^T = K @ Q^T tiles ([k part, q free]) so softmax needs no
    transposes. Score k-tiles land in PAIRS in one [128, 2, QC] fp32 PSUM
    tile (2 banks) but are exp'd PER TILE: a single 1024-wide cross-bank
    ACT exp measured ~+30 us/kernel on HW vs two 512-wide ones, despite the
    cost model preferring it. Causal handling: skip k-tiles above the
    diagonal, col-restrict the st matmuls / exps / ctx matmuls / acc adds
    on diagonal tiles to the live columns, and one [128,128] triangular
    fp16 mask for the 128-wide mixed strip of each diagonal tile. Row sums:
    est tiles accumulate elementwise on the DVE (fp16 2x), then ONE
    ones-matmul partition-reduce per head into a shared [97, QC] sums bank
    (rows 0/32/64/96: main/adapter x head -- 32-aligned output bases).
  - softmax denominators are read back (DVE reciprocals) IMMEDIATELY after
    each head's k-loop: the sums bank shares a pool with out-proj PSUM
    tiles, and freeing it early unblocks the next chunk's allocations.
  - adapter path (L=10) folded into the main ctx PSUM accumulation:
    ct = (ctx + actx_g * sum/asum) / sum, gate pre-folded into the
    host-scaled adapter-V copy (aptv). 1/asum stays fp32 (asum ~ 6e5 makes
    it fp16-subnormal). The [1,QC] -> [L,QC] and [1,QC] -> [128,QC]
    broadcasts are ones-matmuls on PE + an ACT copy; gpsimd
    partition_broadcast measured FAR slower on HW (Q7 software loop with
    DVE-port contention) even though the cost model likes it.
  - output projection uses only the core's OWN 2-head ct against its
    256-column slice of Wo, producing fp16 partials [d, s]; the cross-core
    sum happens on the host as the unshard step. out_proj(qc) is emitted
    sequentially after combine(qc) -- interleaving it into the next
    chunk's k-loop as "PE filler" measured ~+11 us on HW despite looking
    good in the cost model. In the LAST chunk, head 0's combine chain IS
    deferred into head 1's k-loop (hiding its DVE latency under PE score
    work; HW-neutral, helps the model), and the final out-proj ping-pongs
    over FOUR PSUM slots (2 pools) with a 2-ahead first-head window plus
    4-tile-batched output DMA so the tail runs at PE/DMA rate, not at
    PSUM-evacuation-copy + DMA-issue latency.
PSUM banks (8): big x2 (score-pairs / merged q+k proj / v proj / adapter
projections / tail out-proj; [128,2,QC] fp32 slots = 2 banks each), ctx x2,
accps x2 (sums / adapter scores / out-proj accumulators).
Host side: weight slicing/transposes/casts, RoPE tables from position_ids,
partial-sum + transpose.
"""

import math
import numpy as np

import concourse.bass as bass
import concourse.bacc as bacc
import concourse.mybir as mybir
import concourse.tile as tile
from concourse.bass_utils import run_bass_kernel_spmd

F16 = mybir.dt.float16
BF16 = mybir.dt.bfloat16
F32 = mybir.dt.float32
NP_F16 = mybir.dt.np(F16)
NP_BF16 = mybir.dt.np(BF16)


class Cfg:
    def __init__(self, s=2048, d=2048, L=10, n_cores=8, n_heads=16, rope_base=10000.0):
        self.s, self.d, self.L = s, d, L
        self.n_cores = n_cores
        self.n_heads = n_heads
        self.rope_base = rope_base
        self.hd = 128                      # head dim (fixed)
        self.hpc = n_heads // n_cores      # heads per core
        self.dh = self.hpc * self.hd       # local head-dim cols per core
        self.nd = d // 128                 # contraction chunks
        self.QC = 512                      # q-chunk width
        self.ns = s // self.QC             # q-chunks
        self.nst = s // 128                # s tiles (k tiles)
        self.kpq = self.QC // 128          # k-tiles straddling one q-chunk diag
        assert self.hpc * n_cores == n_heads and d % 128 == 0 and s % self.QC == 0
        assert self.kpq % 2 == 0


def build(cfg: Cfg, nrep: int = 1, loop: int | None = None):
    """Build the per-core SPMD graph. Returns compiled nc.
    nrep>1 repeats the whole pipeline unrolled; loop=K wraps ONE copy of the
    pipeline in a Tile For_i hardware loop executing K times (one NEFF-sized
    body, K x the work) -- used for low-noise marginal-time HW measurement."""
    c = cfg
    nc = bacc.Bacc(None, target_bir_lowering=False, num_devices=c.n_cores)

    # ---------------- external I/O (per-core shards) ----------------
    xt_d = nc.dram_tensor("xt", [c.d, c.s], F16, kind="ExternalInput")
    wqk_d = nc.dram_tensor("wqk", [c.d, 2, c.dh], F16, kind="ExternalInput")
    wvt_d = nc.dram_tensor("wvt", [c.d, c.dh], F16, kind="ExternalInput")
    wot_d = nc.dram_tensor("wot", [c.dh, c.d], F16, kind="ExternalInput")
    apt_d = nc.dram_tensor("apt", [c.d, c.L], F16, kind="ExternalInput")
    aptv_d = nc.dram_tensor("aptv", [c.d, c.L], F16, kind="ExternalInput")
    cos_d = nc.dram_tensor("cost", [128, c.s], F16, kind="ExternalInput")
    sin_d = nc.dram_tensor("sint", [128, c.s], F16, kind="ExternalInput")
    out_d = nc.dram_tensor("out", [c.d, c.s], F16, kind="ExternalOutput")

    # single [128,128] lower-triangular mask: tri[k, q] = 1 if k <= q.
    # Diagonal k-tile j of a chunk only mixes masked/unmasked inside a
    # 128-wide column strip; columns below it are handled by col-restricted
    # reads, columns above are fully unmasked.
    kk = np.arange(128)[:, None]
    qq = np.arange(128)[None, :]
    tri_np = (kk <= qq).astype(NP_F16)  # [128, 128]
    tri_d = nc.inline_tensor(tri_np, name="tri")

    scale_s = 1.0 / math.sqrt(c.hd)        # main attention scale
    scale_a = 1.0 / math.sqrt(c.n_heads)   # adapter scale (faithful to ref)

    EXP = mybir.ActivationFunctionType.Exp
    ADD = mybir.AluOpType.add
    MUL = mybir.AluOpType.mult

    with tile.TileContext(nc) as tc:
        with (
            tc.tile_pool(name="bigsb", bufs=1) as bigp,
            tc.tile_pool(name="persist", bufs=1) as pp,
            tc.tile_pool(name="work", bufs=3) as wp,
            tc.tile_pool(name="psum", bufs=1, space="PSUM") as psp,
        ):
            if nrep == 0:
                # timing baseline: touch every input (the terminal only ships
                # buffers the NEFF references) but do ~zero device work
                z = pp.tile([1, 128], F16, tag="z")
                for i, t in enumerate((xt_d, wqk_d, wvt_d, wot_d,
                                       apt_d, aptv_d, cos_d, sin_d)):
                    nc.sync.dma_start(z[0:1, 8 * i:8 * i + 8], t[0:1, 0:8])
                zo = pp.tile([1, 64], F16, tag="zo")
                nc.gpsimd.memset(zo[:], 0.0)
                nc.sync.dma_start(out_d[0:1, 0:64], zo[:])
            import contextlib

            def rep_ctx():
                if loop is not None:
                    return tc.For_i(0, loop, 1)
                return contextlib.nullcontext()

            for _rep in range(nrep if loop is None else 1):
              with rep_ctx():
                # ---------- loads (q/k weights + chunk0, t-sliced, first) ----
                # q and k weights interleaved per t-slice so one DMA
                # stream delivers them in exactly the consumption order
                wqk = pp.tile([128, c.nd, 2, c.dh], F16, tag="wqk")
                wqk_r = wqk_d.rearrange("(t p) two m -> p t two m", p=128)
                xt = bigp.tile([128, c.nd, c.s], F16, tag="big")
                xt_r = xt_d.rearrange("(t p) s -> p t s", p=128)
                # t-sliced startup loads, arrival order == the t-interleaved
                # consumption order of proj_chunk(0): wq/wk ride the SP HWDGE
                # ring while chunk-0 x slices ride the otherwise-idle ACT ring
                bounds = [b for b in (0, 1, 3, 6, 10, 13, c.nd) if b <= c.nd]
                bounds = sorted(set(bounds + [c.nd]))
                cos_t = pp.tile([128, c.s], F16, tag="cos")
                sin_t = pp.tile([128, c.s], F16, tag="sin")
                for i in range(len(bounds) - 1):
                    ts = slice(bounds[i], bounds[i + 1])
                    nc.sync.dma_start(wqk[:, ts, :, :], wqk_r[:, ts, :, :])
                    nc.scalar.dma_start(xt[:, ts, 0:c.QC], xt_r[:, ts, 0:c.QC])
                nc.sync.dma_start(cos_t[:, 0:c.QC], cos_d[:, 0:c.QC])
                nc.sync.dma_start(sin_t[:, 0:c.QC], sin_d[:, 0:c.QC])
                # dummy exp: pulls the ACT function-table load into the
                # startup DMA window instead of stalling the first real copy
                warm = wp.tile([1, 2], F32, tag="warm", bufs=1)
                nc.vector.memset(warm[:], 0.0)
                nc.scalar.activation(warm[:], warm[:], EXP, scale=1.0)
                tri = pp.tile([128, 128], F16, tag="tri")
                nc.sync.dma_start(tri[:], tri_d[:])
                # load order follows first-use time: wvt feeds the chunk-0
                # v-projection (~19 us) BEFORE adapter_kv needs apt (~21 us)
                wvt = pp.tile([128, c.nd, c.dh], F16, tag="wvt")
                nc.sync.dma_start(wvt[:], wvt_d.rearrange("(t p) m -> p t m", p=128))
                apt = pp.tile([128, c.nd, c.L], F16, tag="apt")
                nc.sync.dma_start(apt[:], apt_d.rearrange("(t p) m -> p t m", p=128))
                aptv = pp.tile([128, c.nd, c.L], F16, tag="aptv")
                nc.sync.dma_start(aptv[:], aptv_d.rearrange("(t p) m -> p t m", p=128))
                if c.s > c.QC:
                    sl = slice(c.QC, c.s)
                    nc.sync.dma_start(cos_t[:, sl], cos_d[:, sl])
                    nc.sync.dma_start(sin_t[:, sl], sin_d[:, sl])
                for qc in range(1, c.ns):
                    sl = slice(qc * c.QC, (qc + 1) * c.QC)
                    nc.sync.dma_start(xt[:, :, sl], xt_r[:, :, sl])
                wot = pp.tile([128, c.hpc, c.d], F16, tag="wot")
                nc.sync.dma_start(wot[:], wot_d.rearrange("(t p) m -> p t m", p=128))
                # all-ones: column [:, 0:1] is the row-sum lhsT
                ones_t = pp.tile([128, 128], F16, tag="ones_t")
                nc.gpsimd.memset(ones_t[:], 1.0)

                # ---------- persistent intermediates ----------
                qrot = [pp.tile([128, c.s], F16, tag=f"qrot{h}", name=f"qrot{h}")
                        for h in range(c.hpc)]
                krot = [pp.tile([128, c.s], F16, tag=f"krot{h}", name=f"krot{h}")
                        for h in range(c.hpc)]
                v_sb = pp.tile([128, c.nst, c.dh], F16, tag="v")
                akt = pp.tile([128, c.hpc, c.L], F16, tag="akt")
                av_sb = pp.tile([c.L, c.dh], F16, tag="av")

                def rope_copy(src_ps, on_act):
                    # PSUM -> SBUF evacuation, FIRST so the projection bank
                    # frees after one short copy instead of a full TT chain;
                    # half the copies ride the otherwise-idle ACT engine
                    qf = wp.tile([128, c.QC], F16, tag="qf", bufs=4)
                    if on_act:
                        nc.scalar.copy(qf[:], src_ps[:])
                    else:
                        nc.vector.tensor_copy(qf[:], src_ps[:])
                    return qf

                def rope_tt(dst, qf, sl):
                    # dst[0:64]   = src[0:64]*cos[0:64] - src[64:]*sin[0:64]
                    # dst[64:128] = src[64:]*cos[64:]   + src[0:64]*sin[64:]
                    # sin_t is pre-signed on host: rows 0:64 hold -sin, and
                    # halves are swapped so each TT's two SBUF inputs share a
                    # base partition (HW requirement)
                    t2 = wp.tile([128, c.QC], F16, tag="tmp", bufs=6)
                    nc.vector.tensor_tensor(t2[0:64], qf[64:128],
                                            sin_t[64:128, sl], MUL)
                    nc.vector.tensor_tensor(t2[64:128], qf[0:64],
                                            sin_t[0:64, sl], MUL)
                    t1 = wp.tile([128, c.QC], F16, tag="tmp", bufs=6)
                    nc.vector.tensor_tensor(t1[:], qf[:], cos_t[:, sl], MUL)
                    nc.vector.tensor_tensor(dst, t1[:], t2[:], ADD)

                def proj_chunk(qc):
                    sl = slice(qc * c.QC, (qc + 1) * c.QC)
                    if qc == 0:
                        # t OUTERMOST (4 open accumulation groups: q+k x 2
                        # heads) so chunk-0 consumption matches the startup
                        # DMA arrival order t-slice by t-slice
                        qk_ps = [psp.tile([128, 2, c.QC], F32, tag="big",
                                          bufs=2, name=f"qk{qc}_{h}")
                                 for h in range(c.hpc)]
                        for t in range(c.nd):
                            for h in range(c.hpc):
                                hsl = slice(h * 128, (h + 1) * 128)
                                nc.tensor.matmul(qk_ps[h][:, 0, :],
                                                 wqk[:, t, 0, hsl], xt[:, t, sl],
                                                 start=(t == 0),
                                                 stop=(t == c.nd - 1))
                                nc.tensor.matmul(qk_ps[h][:, 1, :],
                                                 wqk[:, t, 1, hsl], xt[:, t, sl],
                                                 start=(t == 0),
                                                 stop=(t == c.nd - 1))
                        qfs = []
                        for h in range(c.hpc):
                            qfs.append(rope_copy(qk_ps[h][:, 0, :], False))
                            qfs.append(rope_copy(qk_ps[h][:, 1, :], True))
                        for h in range(c.hpc):
                            rope_tt(qrot[h][:, sl], qfs[2 * h], sl)
                            rope_tt(krot[h][:, sl], qfs[2 * h + 1], sl)
                    else:
                        for h in range(c.hpc):
                            hsl = slice(h * 128, (h + 1) * 128)
                            qk_ps = psp.tile([128, 2, c.QC], F32, tag="big",
                                             bufs=2, name=f"qk{qc}_{h}")
                            for t in range(c.nd):
                                nc.tensor.matmul(qk_ps[:, 0, :], wqk[:, t, 0, hsl],
                                                 xt[:, t, sl], start=(t == 0),
                                                 stop=(t == c.nd - 1))
                            for t in range(c.nd):
                                nc.tensor.matmul(qk_ps[:, 1, :], wqk[:, t, 1, hsl],
                                                 xt[:, t, sl], start=(t == 0),
                                                 stop=(t == c.nd - 1))
                            qf_q = rope_copy(qk_ps[:, 0, :], False)
                            qf_k = rope_copy(qk_ps[:, 1, :], True)
                            rope_tt(qrot[h][:, sl], qf_q, sl)
                            rope_tt(krot[h][:, sl], qf_k, sl)
                    for st in range(c.kpq):
                        gst = qc * c.kpq + st
                        ssl = slice(gst * 128, (gst + 1) * 128)
                        # accps pool: free during every proj window, so the
                        # v matmuls never wait on the rope copies that hold
                        # the big slots
                        v_ps = psp.tile([128, c.dh], F32, tag="accps", bufs=2,
                                        name=f"v{qc}_{st}")
                        for t in range(c.nd):
                            nc.tensor.matmul(v_ps[:], xt[:, t, ssl], wvt[:, t, :],
                                             start=(t == 0), stop=(t == c.nd - 1))
                        nc.scalar.copy(v_sb[:, gst, :], v_ps[:])

                def adapter_kv():
                    for h in range(c.hpc):
                        hsl = slice(h * 128, (h + 1) * 128)
                        a_ps = psp.tile([128, c.L], F32, tag="big", bufs=2)
                        for t in range(c.nd):
                            nc.tensor.matmul(a_ps[:], wqk[:, t, 1, hsl], apt[:, t, :],
                                             start=(t == 0), stop=(t == c.nd - 1))
                        nc.scalar.copy(akt[:, h, :], a_ps[:])
                    av_ps = psp.tile([c.L, c.dh], F32, tag="big", bufs=2)
                    for t in range(c.nd):
                        nc.tensor.matmul(av_ps[:], aptv[:, t, :], wvt[:, t, :],
                                         start=(t == 0), stop=(t == c.nd - 1))
                    nc.scalar.copy(av_sb[:], av_ps[:])

                def scores_head(qc, h, state, filler, sums_box):
                    sl = slice(qc * c.QC, (qc + 1) * c.QC)
                    nkt = qc * c.kpq + c.kpq  # causal: k-tiles 0..nkt-1
                    npair = nkt // 2
                    hsl = slice(h * 128, (h + 1) * 128)
                    ctx_ps = psp.tile([128, c.QC], F32, tag="ctx", bufs=2,
                                      name=f"ctx{qc}_{h}")

                    def pair_mm(m):
                        # two k-tiles of S^T into one 2-bank fp32 tile so
                        # the exp below covers 1024 columns in one op; on
                        # diagonal tiles only the live columns are computed
                        ps = psp.tile([128, 2, c.QC], F32, tag="big", bufs=2,
                                      name=f"st{qc}_{h}_{m}")
                        for i in (0, 1):
                            kt = 2 * m + i
                            j = kt - qc * c.kpq
                            lo = 128 * j if j > 0 else 0
                            ksl = slice(kt * 128, (kt + 1) * 128)
                            nc.tensor.matmul(
                                ps[:, i, lo:], krot[h][:, ksl],
                                qrot[h][:, qc * c.QC + lo:(qc + 1) * c.QC],
                                start=True, stop=True)
                        return ps

                    acc = wp.tile([128, c.QC], F16, tag="acc", bufs=2,
                                  name=f"acc{qc}_{h}")
                    st_q = [pair_mm(m) for m in range(min(2, npair))]
                    for m in range(npair):
                        st_cur = st_q.pop(0)
                        if m + 2 < npair:
                            st_q.append(pair_mm(m + 2))
                        est = wp.tile([128, 2, c.QC], F16, tag="est", bufs=6)
                        for i in (0, 1):
                            j = 2 * m + i - qc * c.kpq
                            lo = 128 * j if j > 0 else 0
                            nc.scalar.activation(est[:, i, lo:],
                                                 st_cur[:, i, lo:], EXP,
                                                 scale=scale_s)
                        for i in (0, 1):
                            kt = 2 * m + i
                            j = kt - qc * c.kpq
                            lo = 128 * j if j > 0 else 0
                            if j >= 0:
                                # mixed strip of the diagonal tile
                                nc.vector.tensor_tensor(
                                    est[:, i, 128 * j:128 * j + 128],
                                    est[:, i, 128 * j:128 * j + 128],
                                    tri[:], MUL)
                            nc.tensor.matmul(ctx_ps[:, lo:], v_sb[:, kt, hsl],
                                             est[:, i, lo:],
                                             start=(kt == 0), stop=False)
                            # elementwise est accumulation on the DVE;
                            # values stay O(30) so fp16 is safe
                            if kt == 0:
                                nc.vector.tensor_copy(acc[:], est[:, 0, :])
                            else:
                                nc.vector.tensor_tensor(
                                    acc[:, lo:], acc[:, lo:], est[:, i, lo:],
                                    ADD)
                        filler(1)
                    # adapter attention (no rope on adapter k, 1/sqrt(H))
                    ast_ps = psp.tile([c.L, c.QC], F32, tag="accps", bufs=2,
                                      name=f"ast{qc}_{h}")
                    nc.tensor.matmul(ast_ps[:], akt[:, h, :], qrot[h][:, sl],
                                     start=True, stop=True)
                    aest = wp.tile([c.L, c.QC], BF16, tag="aest", bufs=2,
                                   name=f"aest{qc}_{h}")
                    nc.scalar.activation(aest[:], ast_ps[:], EXP, scale=scale_a)
                    # per-head softmax denominators, one shared PSUM bank:
                    # rows 64h (main) and 64h+32 (adapter) -- matmul outputs
                    # must start at a 32-aligned partition
                    if sums_box[0] is None:
                        sums_box[0] = psp.tile([97, c.QC], F32, tag="accps",
                                               bufs=2, name=f"sums{qc}")
                    sums = sums_box[0]
                    nc.tensor.matmul(sums[64 * h:64 * h + 1, :],
                                     ones_t[:, 0:1], acc[:],
                                     start=True, stop=True,
                                     tile_position=(0, 64 * h))
                    nc.tensor.matmul(sums[64 * h + 32:64 * h + 33, :],
                                     ones_t[0:c.L, 0:1], aest[:],
                                     start=True, stop=True,
                                     tile_position=(0, 64 * h + 32))
                    # ---- early denominator readback (DVE): frees the sums
                    # bank and shortens the combine critical path
                    r1 = wp.tile([1, c.QC], F16, tag="r1", bufs=4,
                                 name=f"r1_{qc}_{h}")
                    ra32 = wp.tile([1, c.QC], F32, tag="ra32", bufs=2,
                                   name=f"ra32_{qc}_{h}")
                    f2 = wp.tile([1, c.QC], F16, tag="f2", bufs=2,
                                 name=f"f2_{qc}_{h}")
                    # adapter sums reach ~6e5, so 1/asum is fp16-SUBNORMAL;
                    # that reciprocal must stay fp32 (f2 = sum/asum itself
                    # is fp16-safe). 1/sum is ~1e-4..1 -> fp16 fine.
                    with nc.allow_low_precision(reason="1/softmax-sum fp16"):
                        nc.vector.reciprocal(r1[:], sums[64 * h:64 * h + 1])
                        nc.vector.reciprocal(ra32[:],
                                             sums[64 * h + 32:64 * h + 33])
                        nc.vector.tensor_tensor(f2[:], sums[64 * h:64 * h + 1],
                                                ra32[:], MUL)
                    state[h] = (ctx_ps, aest, r1, f2)

                def combine_head(qc, h, state, ct_tiles):
                    # combine: ct = (ctx + actx_g*sum/asum)/sum  (gate is
                    # pre-folded into av via the host-scaled aptv). The
                    # [1,QC] -> [L,QC] / [1,QC] -> [128,QC] broadcasts are
                    # ones-matmuls (216 ns on PE); gpsimd partition_broadcast
                    # measured FAR slower on HW (software Q7 loop + DVE-port
                    # contention) despite the cost model liking it. PSUM for
                    # them comes from the "big" pool, whose slots recycle
                    # fast (score-pair exps), so the last chunk's deferred
                    # combine cannot deadlock on the sums bank.
                    hsl = slice(h * 128, (h + 1) * 128)
                    ctx_ps, aest, r1, f2 = state[h]
                    f10_ps = psp.tile([c.L, c.QC], F32, tag="big", bufs=2,
                                      name=f"f10_{qc}_{h}")
                    nc.tensor.matmul(f10_ps[:], ones_t[0:1, 0:c.L], f2[:],
                                     start=True, stop=True)
                    aest2 = wp.tile([c.L, c.QC], F16, tag="aest2", bufs=2,
                                    name=f"aest2_{qc}_{h}")
                    nc.vector.tensor_tensor(aest2[:], aest[:], f10_ps[:], MUL)
                    nc.tensor.matmul(ctx_ps[:], av_sb[:, hsl], aest2[:],
                                     start=False, stop=True)
                    rcb_ps = psp.tile([128, c.QC], F32, tag="big", bufs=2,
                                      name=f"rcb_{qc}_{h}")
                    nc.tensor.matmul(rcb_ps[:], ones_t[0:1, :], r1[:],
                                     start=True, stop=True)
                    rcb = wp.tile([128, c.QC], F16, tag="rcb", bufs=2,
                                  name=f"rcbs{qc}_{h}")
                    nc.scalar.copy(rcb[:], rcb_ps[:])
                    ct = wp.tile([128, c.QC], F16, tag="ct", bufs=6,
                                 name=f"ct{qc}_{h}")
                    nc.vector.tensor_tensor(ct[:], ctx_ps[:], rcb[:], MUL)
                    ct_tiles[h] = ct

                def out_proj_units(qc, ct_tiles, copy_split=None):
                    # out_pT[do, q] += wot[:, h, do].T @ ct[h]  (local heads
                    # only; cross-core reduction happens on the host). One
                    # emit-closure per dt tile; interleaved into the next
                    # chunk's score loop as PE filler.
                    sl = slice(qc * c.QC, (qc + 1) * c.QC)

                    def unit(dt):
                        def emit():
                            dsl = slice(dt * 128, (dt + 1) * 128)
                            o_ps = psp.tile([128, c.QC], F32, tag="accps",
                                            bufs=2, name=f"o_ps{qc}_{dt}")
                            for h in range(c.hpc):
                                nc.tensor.matmul(o_ps[:], wot[:, h, dsl],
                                                 ct_tiles[h][:],
                                                 start=(h == 0),
                                                 stop=(h == c.hpc - 1))
                            o_sb = wp.tile([128, c.QC], F16, tag="osb", bufs=6,
                                           name=f"o_sb{qc}_{dt}")
                            if copy_split is None:
                                on_dve = dt % 2 == 1
                            else:
                                on_dve = dt < copy_split
                            if on_dve:
                                nc.vector.tensor_copy(o_sb[:], o_ps[:])
                            else:
                                nc.scalar.copy(o_sb[:], o_ps[:])
                            nc.sync.dma_start(out_d[dsl, sl], o_sb[:])
                        return emit
                    return [unit(dt) for dt in range(c.nd)]

                def out_proj_tail(qc, ct_tiles):
                    # final out-proj: 4 PSUM slots (accps + big pools) and a
                    # 2-ahead first-head window so the PE never waits on the
                    # PSUM-evacuation copies. Output DMA batched 4 dt-tiles
                    # per transfer on the SP ring (one issue slot, line-rate)
                    sl = slice(qc * c.QC, (qc + 1) * c.QC)
                    out_r = out_d.rearrange("(t p) s -> p t s", p=128)
                    o_ps = {}
                    grp = 4
                    o_sb4 = None

                    def open_dt(dt):
                        tag = "big" if dt % 2 == 0 else "accps"
                        ps = psp.tile([128, c.QC], F32, tag=tag, bufs=2,
                                      name=f"o_ps{qc}_{dt}")
                        dsl = slice(dt * 128, (dt + 1) * 128)
                        nc.tensor.matmul(ps[:], wot[:, 0, dsl], ct_tiles[0][:],
                                         start=True, stop=(c.hpc == 1))
                        o_ps[dt] = ps

                    # group bounds: big batches first, tiny last ones so
                    # the final transfer lands right after the final copy
                    cuts = list(range(grp, c.nd, grp)) + [c.nd]
                    gstart = 0
                    for dt in range(min(2, c.nd)):
                        open_dt(dt)
                    for dt in range(c.nd):
                        ps = o_ps.pop(dt)
                        dsl = slice(dt * 128, (dt + 1) * 128)
                        for h in range(1, c.hpc):
                            nc.tensor.matmul(ps[:], wot[:, h, dsl],
                                             ct_tiles[h][:],
                                             start=False, stop=(h == c.hpc - 1))
                        if dt + 2 < c.nd:
                            open_dt(dt + 2)
                        gend = min(cc for cc in cuts if cc > dt)
                        if dt == gstart:
                            o_sb4 = wp.tile([128, gend - gstart, c.QC], F16,
                                            tag="osb4", bufs=2,
                                            name=f"o_sb4_{qc}_{dt}")
                        if dt % 2 == 1:
                            nc.vector.tensor_copy(o_sb4[:, dt - gstart, :], ps[:])
                        else:
                            nc.scalar.copy(o_sb4[:, dt - gstart, :], ps[:])
                        if dt == gend - 1:
                            nc.sync.dma_start(out_r[:, gstart:gend, sl],
                                              o_sb4[:])
                            gstart = gend

                # ---------- fused pipeline ----------
                proj_chunk(0)
                adapter_kv()
                pending = []

                def filler(n):
                    for _ in range(n):
                        if pending:
                            pending.pop(0)()

                for qc in range(c.ns):
                    last = qc + 1 == c.ns
                    state, ct_tiles = {}, {}
                    sums_box = [None]
                    for h in range(c.hpc):
                        scores_head(qc, h, state, filler, sums_box)
                        if last and h + 1 < c.hpc:
                            # hide head h's combine chain under head h+1's
                            # k-loop (drained via the filler)
                            hh = h

                            def comb():
                                combine_head(qc, hh, state, ct_tiles)
                            pending.insert(0, comb)
                    while pending:
                        pending.pop(0)()
                    if not last:
                        proj_chunk(qc + 1)
                        for h in range(c.hpc):
                            combine_head(qc, h, state, ct_tiles)
                        for u in out_proj_units(qc, ct_tiles):
                            u()
                    else:
                        if c.hpc == 1:
                            combine_head(qc, 0, state, ct_tiles)
                        else:
                            combine_head(qc, c.hpc - 1, state, ct_tiles)
                        out_proj_tail(qc, ct_tiles)

    nc.compile()
    return nc


def make_in_maps(cfg, hidden_states, Wq, Wk, Wv, Wo, adaption_prompt,
                 adaption_gate, position_ids):
    """Host-side sharding: slice/transpose/cast per core + RoPE tables."""
    c = cfg
    x = np.asarray(hidden_states, np.float32)[0]          # [s, d]
    xt = np.ascontiguousarray(x.T).astype(NP_F16)         # [d, s]
    ap = np.asarray(adaption_prompt, np.float32)[0]       # [L, d]
    apt = np.ascontiguousarray(ap.T).astype(NP_F16)       # [d, L]
    gate = float(np.asarray(adaption_gate).reshape(-1)[0])
    aptv = np.ascontiguousarray(gate * ap.T).astype(NP_F16)
    # RoPE tables in the [hd, s] transposed layout; sin pre-signed.
    pos = np.asarray(position_ids).reshape(-1).astype(np.float64)  # [s]
    inv = 1.0 / (c.rope_base ** (np.arange(0, c.hd, 2, dtype=np.float64) / c.hd))
    f = inv[:, None] * pos[None, :]                       # [hd/2, s]
    cos_t = np.concatenate([np.cos(f), np.cos(f)], axis=0).astype(NP_F16)
    sv = np.sin(f)
    # halves swapped: rows 0:64 multiply q[0:64] (+sin, lands in dst[64:]),
    # rows 64:128 multiply q[64:128] (-sin, lands in dst[0:64])
    sin_t = np.concatenate([sv, -sv], axis=0).astype(NP_F16)
    in_maps = []
    for i in range(c.n_cores):
        rs = slice(i * c.dh, (i + 1) * c.dh)
        wq_t = np.asarray(Wq, np.float32)[rs, :].T.astype(NP_F16)   # [d, dh]
        wk_t = np.asarray(Wk, np.float32)[rs, :].T.astype(NP_F16)
        wqk = np.ascontiguousarray(np.stack([wq_t, wk_t], axis=1))  # [d, 2, dh]
        in_maps.append({
            "xt": xt,
            "wqk": wqk,
            "wvt": np.ascontiguousarray(np.asarray(Wv, np.float32)[rs, :].T).astype(NP_F16),
            "wot": np.ascontiguousarray(np.asarray(Wo, np.float32)[:, rs].T).astype(NP_F16),
            "apt": apt,
            "aptv": aptv,
            "cost": cos_t,
            "sint": sin_t,
        })
    return in_maps


def assemble_output(cfg, results):
    acc = np.zeros((cfg.d, cfg.s), np.float32)
    for r in results:
        acc += np.asarray(r["out"], np.float32)           # per-core partial [d, s]
    return np.ascontiguousarray(acc.T)[None]              # [1, s, d]


_NC_CACHE = {}


def run(inputs, cfg=None, trace=False):
    cfg = cfg or Cfg()
    key = (cfg.s, cfg.d, cfg.L, cfg.n_cores, cfg.n_heads)
    if key not in _NC_CACHE:
        _NC_CACHE[key] = build(cfg)
    nc = _NC_CACHE[key]
    in_maps = make_in_maps(cfg, **inputs)
    res = run_bass_kernel_spmd(nc, in_maps, core_ids=list(range(cfg.n_cores)),
                               trace=trace)
    out = assemble_output(cfg, res.results)
    return out, res


def kernel(**inputs) -> np.ndarray:
    out, _ = run(inputs)
    return out.astype(np.float32)


# revision 22
# speedup vs baseline: 1.1123x; 1.0007x over previous
"""AdaptedAttention (llama + adaption-prompt) on 8 TRN2 NeuronCores.

Sharding: tensor-parallel over heads (2 heads/core), zero device collectives.
Per core (everything on-chip fp16; PSUM accumulation fp32):
  - qT/kT/V projections for its 2 heads from fp16, pre-transposed X
    (all matmuls contract over d, so X lives on-chip as X.T [d part, s free]).
    The q/k projection t-loop is OUTERMOST (4 open PSUM accumulation groups:
    q+k x 2 heads) so chunk-0's PE consumption order matches the startup DMA
    arrival order t-slice by t-slice -- PE starts ~1 us after launch and
    streams at the DMA rate instead of stalling on late weight slices.
  - RoPE in the [hd, s] layout with HOST-precomputed fp16 cos/sin tables
    (sin pre-signed AND half-swapped so each DVE TensorTensor's two SBUF
    inputs share a base partition -- a HW requirement)
  - attention as S^T = K @ Q^T tiles ([k part, q free]) so softmax needs no
    transposes. Score k-tiles land in PAIRS in one [128, 2, QC] fp32 PSUM
    tile (2 banks) but are exp'd PER TILE: a single 1024-wide cross-bank
    ACT exp measured ~+30 us/kernel on HW vs two 512-wide ones, despite the
    cost model preferring it. Causal handling: skip k-tiles above the
    diagonal, col-restrict the st matmuls / exps / ctx matmuls / acc adds
    on diagonal tiles to the live columns, and one [128,128] triangular
    fp16 mask for the 128-wide mixed strip of each diagonal tile. Row sums:
    est tiles accumulate elementwise on the DVE (fp16 2x), then ONE
    ones-matmul partition-reduce per head into a shared [97, QC] sums bank
    (rows 0/32/64/96: main/adapter x head -- 32-aligned output bases).
  - softmax denominators are read back (DVE reciprocals) IMMEDIATELY after
    each head's k-loop: the sums bank shares a pool with out-proj PSUM
    tiles, and freeing it early unblocks the next chunk's allocations.
  - adapter path (L=10) folded into the main ctx PSUM accumulation:
    ct = (ctx + actx_g * sum/asum) / sum, gate pre-folded into the
    host-scaled adapter-V copy (aptv). 1/asum stays fp32 (asum ~ 6e5 makes
    it fp16-subnormal). The [1,QC] -> [L,QC] and [1,QC] -> [128,QC]
    broadcasts are ones-matmuls on PE + an ACT copy; gpsimd
    partition_broadcast measured FAR slower on HW (Q7 software loop with
    DVE-port contention) even though the cost model likes it.
  - output projection uses only the core's OWN 2-head ct against its
    256-column slice of Wo, producing fp16 partials [d, s]; the cross-core
    sum happens on the host as the unshard step. out_proj(qc) is emitted
    sequentially after combine(qc) -- interleaving it into the next
    chunk's k-loop as "PE filler" measured ~+11 us on HW despite looking
    good in the cost model. In the LAST chunk, head 0's combine chain IS
    deferred into head 1's k-loop (hiding its DVE latency under PE score
    work; HW-neutral, helps the model), and the final out-proj ping-pongs
    over FOUR PSUM slots (2 pools) with a 2-ahead first-head window plus
    4-tile-batched output DMA so the tail runs at PE/DMA rate, not at
    PSUM-evacuation-copy + DMA-issue latency.
PSUM banks (8): big x2 (score-pairs / merged q+k proj / v proj / adapter
projections / tail out-proj; [128,2,QC] fp32 slots = 2 banks each), ctx x2,
accps x2 (sums / adapter scores / out-proj accumulators).
Host side: weight slicing/transposes/casts, RoPE tables from position_ids,
partial-sum + transpose.
"""

import math
import numpy as np

import concourse.bass as bass
import concourse.bacc as bacc
import concourse.mybir as mybir
import concourse.tile as tile
from concourse.bass_utils import run_bass_kernel_spmd

F16 = mybir.dt.float16
BF16 = mybir.dt.bfloat16
F32 = mybir.dt.float32
NP_F16 = mybir.dt.np(F16)
NP_BF16 = mybir.dt.np(BF16)


class Cfg:
    def __init__(self, s=2048, d=2048, L=10, n_cores=8, n_heads=16, rope_base=10000.0):
        self.s, self.d, self.L = s, d, L
        self.n_cores = n_cores
        self.n_heads = n_heads
        self.rope_base = rope_base
        self.hd = 128                      # head dim (fixed)
        self.hpc = n_heads // n_cores      # heads per core
        self.dh = self.hpc * self.hd       # local head-dim cols per core
        self.nd = d // 128                 # contraction chunks
        self.QC = 512                      # q-chunk width
        self.ns = s // self.QC             # q-chunks
        self.nst = s // 128                # s tiles (k tiles)
        self.kpq = self.QC // 128          # k-tiles straddling one q-chunk diag
        assert self.hpc * n_cores == n_heads and d % 128 == 0 and s % self.QC == 0
        assert self.kpq % 2 == 0


def build(cfg: Cfg, nrep: int = 1, loop: int | None = None):
    """Build the per-core SPMD graph. Returns compiled nc.
    nrep>1 repeats the whole pipeline unrolled; loop=K wraps ONE copy of the
    pipeline in a Tile For_i hardware loop executing K times (one NEFF-sized
    body, K x the work) -- used for low-noise marginal-time HW measurement."""
    c = cfg
    nc = bacc.Bacc(None, target_bir_lowering=False, num_devices=c.n_cores)

    # ---------------- external I/O (per-core shards) ----------------
    xt_d = nc.dram_tensor("xt", [c.d, c.s], F16, kind="ExternalInput")
    wqk_d = nc.dram_tensor("wqk", [c.d, 2, c.dh], F16, kind="ExternalInput")
    wvt_d = nc.dram_tensor("wvt", [c.d, c.dh], F16, kind="ExternalInput")
    wot_d = nc.dram_tensor("wot", [c.dh, c.d], F16, kind="ExternalInput")
    apt_d = nc.dram_tensor("apt", [c.d, c.L], F16, kind="ExternalInput")
    aptv_d = nc.dram_tensor("aptv", [c.d, c.L], F16, kind="ExternalInput")
    cos_d = nc.dram_tensor("cost", [128, c.s], F16, kind="ExternalInput")
    sin_d = nc.dram_tensor("sint", [128, c.s], F16, kind="ExternalInput")
    out_d = nc.dram_tensor("out", [c.d, c.s], F16, kind="ExternalOutput")

    # single [128,128] lower-triangular mask: tri[k, q] = 1 if k <= q.
    # Diagonal k-tile j of a chunk only mixes masked/unmasked inside a
    # 128-wide column strip; columns below it are handled by col-restricted
    # reads, columns above are fully unmasked.
    kk = np.arange(128)[:, None]
    qq = np.arange(128)[None, :]
    tri_np = (kk <= qq).astype(NP_F16)  # [128, 128]
    tri_d = nc.inline_tensor(tri_np, name="tri")

    scale_s = 1.0 / math.sqrt(c.hd)        # main attention scale
    scale_a = 1.0 / math.sqrt(c.n_heads)   # adapter scale (faithful to ref)

    EXP = mybir.ActivationFunctionType.Exp
    ADD = mybir.AluOpType.add
    MUL = mybir.AluOpType.mult

    with tile.TileContext(nc) as tc:
        with (
            tc.tile_pool(name="bigsb", bufs=1) as bigp,
            tc.tile_pool(name="persist", bufs=1) as pp,
            tc.tile_pool(name="work", bufs=3) as wp,
            tc.tile_pool(name="psum", bufs=1, space="PSUM") as psp,
        ):
            if nrep == 0:
                # timing baseline: touch every input (the terminal only ships
                # buffers the NEFF references) but do ~zero device work
                z = pp.tile([1, 128], F16, tag="z")
                for i, t in enumerate((xt_d, wqk_d, wvt_d, wot_d,
                                       apt_d, aptv_d, cos_d, sin_d)):
                    nc.sync.dma_start(z[0:1, 8 * i:8 * i + 8], t[0:1, 0:8])
                zo = pp.tile([1, 64], F16, tag="zo")
                nc.gpsimd.memset(zo[:], 0.0)
                nc.sync.dma_start(out_d[0:1, 0:64], zo[:])
            import contextlib

            def rep_ctx():
                if loop is not None:
                    return tc.For_i(0, loop, 1, staggered_reset=True)
                return contextlib.nullcontext()

            for _rep in range(nrep if loop is None else 1):
              with rep_ctx():
                # ---------- loads (q/k weights + chunk0, t-sliced, first) ----
                # q and k weights interleaved per t-slice so one DMA
                # stream delivers them in exactly the consumption order
                wqk = pp.tile([128, c.nd, 2, c.dh], F16, tag="wqk")
                wqk_r = wqk_d.rearrange("(t p) two m -> p t two m", p=128)
                xt = bigp.tile([128, c.nd, c.s], F16, tag="big")
                xt_r = xt_d.rearrange("(t p) s -> p t s", p=128)
                # t-sliced startup loads, arrival order == the t-interleaved
                # consumption order of proj_chunk(0): wq/wk ride the SP HWDGE
                # ring while chunk-0 x slices ride the otherwise-idle ACT ring
                bounds = [b for b in (0, 1, 3, 6, 10, 13, c.nd) if b <= c.nd]
                bounds = sorted(set(bounds + [c.nd]))
                cos_t = pp.tile([128, c.s], F16, tag="cos")
                sin_t = pp.tile([128, c.s], F16, tag="sin")
                for i in range(len(bounds) - 1):
                    ts = slice(bounds[i], bounds[i + 1])
                    nc.sync.dma_start(wqk[:, ts, :, :], wqk_r[:, ts, :, :])
                    nc.scalar.dma_start(xt[:, ts, 0:c.QC], xt_r[:, ts, 0:c.QC])
                nc.sync.dma_start(cos_t[:, 0:c.QC], cos_d[:, 0:c.QC])
                nc.sync.dma_start(sin_t[:, 0:c.QC], sin_d[:, 0:c.QC])
                # dummy exp: pulls the ACT function-table load into the
                # startup DMA window instead of stalling the first real copy
                warm = wp.tile([1, 2], F32, tag="warm", bufs=1)
                nc.vector.memset(warm[:], 0.0)
                nc.scalar.activation(warm[:], warm[:], EXP, scale=1.0)
                # HAM warm-up: the PE idles ~3 us waiting for the first
                # weight slice; ~28 dummy matmuls on the ones tile keep the
                # PE activity monitor busy so the first REAL matmul burst
                # runs at 2.4 GHz instead of the cold 1.2 GHz half-clock.
                # (scratch PSUM result is consumed by one cheap DVE copy so
                # DCE cannot drop the chain)
                ones_t = pp.tile([128, 128], F16, tag="ones_t")
                nc.gpsimd.memset(ones_t[:], 1.0)
                wu_ps = psp.tile([128, 128], F32, tag="accps", bufs=2,
                                 name="warmup_ps")
                for _w in range(28):
                    nc.tensor.matmul(wu_ps[:], ones_t[:], ones_t[:],
                                     start=True, stop=True)
                wu_sb = wp.tile([1, 2], F16, tag="wusb", bufs=1)
                nc.vector.tensor_copy(wu_sb[:], wu_ps[0:1, 0:2])
                tri = pp.tile([128, 128], F16, tag="tri")
                nc.sync.dma_start(tri[:], tri_d[:])
                # load order follows first-use time: wvt feeds the chunk-0
                # v-projection (~19 us) BEFORE adapter_kv needs apt (~21 us)
                wvt = pp.tile([128, c.nd, c.dh], F16, tag="wvt")
                nc.sync.dma_start(wvt[:], wvt_d.rearrange("(t p) m -> p t m", p=128))
                apt = pp.tile([128, c.nd, c.L], F16, tag="apt")
                nc.sync.dma_start(apt[:], apt_d.rearrange("(t p) m -> p t m", p=128))
                aptv = pp.tile([128, c.nd, c.L], F16, tag="aptv")
                nc.sync.dma_start(aptv[:], aptv_d.rearrange("(t p) m -> p t m", p=128))
                if c.s > c.QC:
                    sl = slice(c.QC, c.s)
                    nc.sync.dma_start(cos_t[:, sl], cos_d[:, sl])
                    nc.sync.dma_start(sin_t[:, sl], sin_d[:, sl])
                for qc in range(1, c.ns):
                    sl = slice(qc * c.QC, (qc + 1) * c.QC)
                    nc.sync.dma_start(xt[:, :, sl], xt_r[:, :, sl])
                wot = pp.tile([128, c.hpc, c.d], F16, tag="wot")
                nc.sync.dma_start(wot[:], wot_d.rearrange("(t p) m -> p t m", p=128))
                # (ones_t allocated above for the HAM warm-up; column
                # [:, 0:1] doubles as the row-sum lhsT)

                # ---------- persistent intermediates ----------
                qrot = [pp.tile([128, c.s], F16, tag=f"qrot{h}", name=f"qrot{h}")
                        for h in range(c.hpc)]
                krot = [pp.tile([128, c.s], F16, tag=f"krot{h}", name=f"krot{h}")
                        for h in range(c.hpc)]
                v_sb = pp.tile([128, c.nst, c.dh], F16, tag="v")
                akt = pp.tile([128, c.hpc, c.L], F16, tag="akt")
                av_sb = pp.tile([c.L, c.dh], F16, tag="av")

                def rope_copy(src_ps, on_act):
                    # PSUM -> SBUF evacuation, FIRST so the projection bank
                    # frees after one short copy instead of a full TT chain;
                    # half the copies ride the otherwise-idle ACT engine
                    qf = wp.tile([128, c.QC], F16, tag="qf", bufs=4)
                    if on_act:
                        nc.scalar.copy(qf[:], src_ps[:])
                    else:
                        nc.vector.tensor_copy(qf[:], src_ps[:])
                    return qf

                def rope_tt(dst, qf, sl):
                    # dst[0:64]   = src[0:64]*cos[0:64] - src[64:]*sin[0:64]
                    # dst[64:128] = src[64:]*cos[64:]   + src[0:64]*sin[64:]
                    # sin_t is pre-signed on host: rows 0:64 hold -sin, and
                    # halves are swapped so each TT's two SBUF inputs share a
                    # base partition (HW requirement)
                    t2 = wp.tile([128, c.QC], F16, tag="tmp", bufs=6)
                    nc.vector.tensor_tensor(t2[0:64], qf[64:128],
                                            sin_t[64:128, sl], MUL)
                    nc.vector.tensor_tensor(t2[64:128], qf[0:64],
                                            sin_t[0:64, sl], MUL)
                    t1 = wp.tile([128, c.QC], F16, tag="tmp", bufs=6)
                    nc.vector.tensor_tensor(t1[:], qf[:], cos_t[:, sl], MUL)
                    nc.vector.tensor_tensor(dst, t1[:], t2[:], ADD)

                def proj_chunk(qc):
                    sl = slice(qc * c.QC, (qc + 1) * c.QC)
                    if qc == 0:
                        # t OUTERMOST (4 open accumulation groups: q+k x 2
                        # heads) so chunk-0 consumption matches the startup
                        # DMA arrival order t-slice by t-slice
                        qk_ps = [psp.tile([128, 2, c.QC], F32, tag="big",
                                          bufs=2, name=f"qk{qc}_{h}")
                                 for h in range(c.hpc)]
                        for t in range(c.nd):
                            for h in range(c.hpc):
                                hsl = slice(h * 128, (h + 1) * 128)
                                nc.tensor.matmul(qk_ps[h][:, 0, :],
                                                 wqk[:, t, 0, hsl], xt[:, t, sl],
                                                 start=(t == 0),
                                                 stop=(t == c.nd - 1))
                                nc.tensor.matmul(qk_ps[h][:, 1, :],
                                                 wqk[:, t, 1, hsl], xt[:, t, sl],
                                                 start=(t == 0),
                                                 stop=(t == c.nd - 1))
                        qfs = []
                        for h in range(c.hpc):
                            qfs.append(rope_copy(qk_ps[h][:, 0, :], False))
                            qfs.append(rope_copy(qk_ps[h][:, 1, :], True))
                        for h in range(c.hpc):
                            rope_tt(qrot[h][:, sl], qfs[2 * h], sl)
                            rope_tt(krot[h][:, sl], qfs[2 * h + 1], sl)
                    else:
                        for h in range(c.hpc):
                            hsl = slice(h * 128, (h + 1) * 128)
                            qk_ps = psp.tile([128, 2, c.QC], F32, tag="big",
                                             bufs=2, name=f"qk{qc}_{h}")
                            for t in range(c.nd):
                                nc.tensor.matmul(qk_ps[:, 0, :], wqk[:, t, 0, hsl],
                                                 xt[:, t, sl], start=(t == 0),
                                                 stop=(t == c.nd - 1))
                            for t in range(c.nd):
                                nc.tensor.matmul(qk_ps[:, 1, :], wqk[:, t, 1, hsl],
                                                 xt[:, t, sl], start=(t == 0),
                                                 stop=(t == c.nd - 1))
                            qf_q = rope_copy(qk_ps[:, 0, :], False)
                            qf_k = rope_copy(qk_ps[:, 1, :], True)
                            rope_tt(qrot[h][:, sl], qf_q, sl)
                            rope_tt(krot[h][:, sl], qf_k, sl)
                    for st in range(c.kpq):
                        gst = qc * c.kpq + st
                        ssl = slice(gst * 128, (gst + 1) * 128)
                        # accps pool: free during every proj window, so the
                        # v matmuls never wait on the rope copies that hold
                        # the big slots
                        v_ps = psp.tile([128, c.dh], F32, tag="accps", bufs=2,
                                        name=f"v{qc}_{st}")
                        for t in range(c.nd):
                            nc.tensor.matmul(v_ps[:], xt[:, t, ssl], wvt[:, t, :],
                                             start=(t == 0), stop=(t == c.nd - 1))
                        nc.scalar.copy(v_sb[:, gst, :], v_ps[:])

                def adapter_kv():
                    for h in range(c.hpc):
                        hsl = slice(h * 128, (h + 1) * 128)
                        a_ps = psp.tile([128, c.L], F32, tag="big", bufs=2)
                        for t in range(c.nd):
                            nc.tensor.matmul(a_ps[:], wqk[:, t, 1, hsl], apt[:, t, :],
                                             start=(t == 0), stop=(t == c.nd - 1))
                        nc.scalar.copy(akt[:, h, :], a_ps[:])
                    av_ps = psp.tile([c.L, c.dh], F32, tag="big", bufs=2)
                    for t in range(c.nd):
                        nc.tensor.matmul(av_ps[:], aptv[:, t, :], wvt[:, t, :],
                                         start=(t == 0), stop=(t == c.nd - 1))
                    nc.scalar.copy(av_sb[:], av_ps[:])

                def scores_head(qc, h, state, filler, sums_box):
                    sl = slice(qc * c.QC, (qc + 1) * c.QC)
                    nkt = qc * c.kpq + c.kpq  # causal: k-tiles 0..nkt-1
                    npair = nkt // 2
                    hsl = slice(h * 128, (h + 1) * 128)
                    ctx_ps = psp.tile([128, c.QC], F32, tag="ctx", bufs=2,
                                      name=f"ctx{qc}_{h}")

                    def pair_mm(m):
                        # two k-tiles of S^T into one 2-bank fp32 tile so
                        # the exp below covers 1024 columns in one op; on
                        # diagonal tiles only the live columns are computed
                        ps = psp.tile([128, 2, c.QC], F32, tag="big", bufs=2,
                                      name=f"st{qc}_{h}_{m}")
                        for i in (0, 1):
                            kt = 2 * m + i
                            j = kt - qc * c.kpq
                            lo = 128 * j if j > 0 else 0
                            ksl = slice(kt * 128, (kt + 1) * 128)
                            nc.tensor.matmul(
                                ps[:, i, lo:], krot[h][:, ksl],
                                qrot[h][:, qc * c.QC + lo:(qc + 1) * c.QC],
                                start=True, stop=True)
                        return ps

                    acc = wp.tile([128, c.QC], F16, tag="acc", bufs=2,
                                  name=f"acc{qc}_{h}")
                    st_q = [pair_mm(m) for m in range(min(2, npair))]
                    for m in range(npair):
                        st_cur = st_q.pop(0)
                        if m + 2 < npair:
                            st_q.append(pair_mm(m + 2))
                        est = wp.tile([128, 2, c.QC], F16, tag="est", bufs=8)
                        for i in (0, 1):
                            j = 2 * m + i - qc * c.kpq
                            lo = 128 * j if j > 0 else 0
                            nc.scalar.activation(est[:, i, lo:],
                                                 st_cur[:, i, lo:], EXP,
                                                 scale=scale_s)
                        for i in (0, 1):
                            kt = 2 * m + i
                            j = kt - qc * c.kpq
                            lo = 128 * j if j > 0 else 0
                            if j >= 0:
                                # mixed strip of the diagonal tile
                                nc.vector.tensor_tensor(
                                    est[:, i, 128 * j:128 * j + 128],
                                    est[:, i, 128 * j:128 * j + 128],
                                    tri[:], MUL)
                            nc.tensor.matmul(ctx_ps[:, lo:], v_sb[:, kt, hsl],
                                             est[:, i, lo:],
                                             start=(kt == 0), stop=False)
                            # elementwise est accumulation on the DVE;
                            # values stay O(30) so fp16 is safe
                            if kt == 0:
                                nc.vector.tensor_copy(acc[:], est[:, 0, :])
                            else:
                                nc.vector.tensor_tensor(
                                    acc[:, lo:], acc[:, lo:], est[:, i, lo:],
                                    ADD)
                        filler(1)
                    # adapter attention (no rope on adapter k, 1/sqrt(H))
                    ast_ps = psp.tile([c.L, c.QC], F32, tag="accps", bufs=2,
                                      name=f"ast{qc}_{h}")
                    nc.tensor.matmul(ast_ps[:], akt[:, h, :], qrot[h][:, sl],
                                     start=True, stop=True)
                    aest = wp.tile([c.L, c.QC], BF16, tag="aest", bufs=2,
                                   name=f"aest{qc}_{h}")
                    nc.scalar.activation(aest[:], ast_ps[:], EXP, scale=scale_a)
                    # per-head softmax denominators, one shared PSUM bank:
                    # rows 64h (main) and 64h+32 (adapter) -- matmul outputs
                    # must start at a 32-aligned partition
                    if sums_box[0] is None:
                        sums_box[0] = psp.tile([97, c.QC], F32, tag="accps",
                                               bufs=2, name=f"sums{qc}")
                    sums = sums_box[0]
                    nc.tensor.matmul(sums[64 * h:64 * h + 1, :],
                                     ones_t[:, 0:1], acc[:],
                                     start=True, stop=True,
                                     tile_position=(0, 64 * h))
                    nc.tensor.matmul(sums[64 * h + 32:64 * h + 33, :],
                                     ones_t[0:c.L, 0:1], aest[:],
                                     start=True, stop=True,
                                     tile_position=(0, 64 * h + 32))
                    # ---- early denominator readback (DVE): frees the sums
                    # bank and shortens the combine critical path
                    r1 = wp.tile([1, c.QC], F16, tag="r1", bufs=4,
                                 name=f"r1_{qc}_{h}")
                    ra32 = wp.tile([1, c.QC], F32, tag="ra32", bufs=2,
                                   name=f"ra32_{qc}_{h}")
                    f2 = wp.tile([1, c.QC], F16, tag="f2", bufs=2,
                                 name=f"f2_{qc}_{h}")
                    # adapter sums reach ~6e5, so 1/asum is fp16-SUBNORMAL;
                    # that reciprocal must stay fp32 (f2 = sum/asum itself
                    # is fp16-safe). 1/sum is ~1e-4..1 -> fp16 fine.
                    with nc.allow_low_precision(reason="1/softmax-sum fp16"):
                        nc.vector.reciprocal(r1[:], sums[64 * h:64 * h + 1])
                        nc.vector.reciprocal(ra32[:],
                                             sums[64 * h + 32:64 * h + 33])
                        nc.vector.tensor_tensor(f2[:], sums[64 * h:64 * h + 1],
                                                ra32[:], MUL)
                    state[h] = (ctx_ps, aest, r1, f2)

                def combine_head(qc, h, state, ct_tiles):
                    # combine: ct = (ctx + actx_g*sum/asum)/sum  (gate is
                    # pre-folded into av via the host-scaled aptv). The
                    # [1,QC] -> [L,QC] / [1,QC] -> [128,QC] broadcasts are
                    # ones-matmuls (216 ns on PE); gpsimd partition_broadcast
                    # measured FAR slower on HW (software Q7 loop + DVE-port
                    # contention) despite the cost model liking it. PSUM for
                    # them comes from the "big" pool, whose slots recycle
                    # fast (score-pair exps), so the last chunk's deferred
                    # combine cannot deadlock on the sums bank.
                    hsl = slice(h * 128, (h + 1) * 128)
                    ctx_ps, aest, r1, f2 = state[h]
                    f10_ps = psp.tile([c.L, c.QC], F32, tag="big", bufs=2,
                                      name=f"f10_{qc}_{h}")
                    nc.tensor.matmul(f10_ps[:], ones_t[0:1, 0:c.L], f2[:],
                                     start=True, stop=True)
                    aest2 = wp.tile([c.L, c.QC], F16, tag="aest2", bufs=2,
                                    name=f"aest2_{qc}_{h}")
                    nc.vector.tensor_tensor(aest2[:], aest[:], f10_ps[:], MUL)
                    nc.tensor.matmul(ctx_ps[:], av_sb[:, hsl], aest2[:],
                                     start=False, stop=True)
                    rcb_ps = psp.tile([128, c.QC], F32, tag="big", bufs=2,
                                      name=f"rcb_{qc}_{h}")
                    nc.tensor.matmul(rcb_ps[:], ones_t[0:1, :], r1[:],
                                     start=True, stop=True)
                    rcb = wp.tile([128, c.QC], F16, tag="rcb", bufs=2,
                                  name=f"rcbs{qc}_{h}")
                    nc.scalar.copy(rcb[:], rcb_ps[:])
                    ct = wp.tile([128, c.QC], F16, tag="ct", bufs=6,
                                 name=f"ct{qc}_{h}")
                    nc.vector.tensor_tensor(ct[:], ctx_ps[:], rcb[:], MUL)
                    ct_tiles[h] = ct

                def out_proj_units(qc, ct_tiles, copy_split=None):
                    # out_pT[do, q] += wot[:, h, do].T @ ct[h]  (local heads
                    # only; cross-core reduction happens on the host). One
                    # emit-closure per dt tile; interleaved into the next
                    # chunk's score loop as PE filler.
                    sl = slice(qc * c.QC, (qc + 1) * c.QC)

                    def unit(dt):
                        def emit():
                            dsl = slice(dt * 128, (dt + 1) * 128)
                            o_ps = psp.tile([128, c.QC], F32, tag="accps",
                                            bufs=2, name=f"o_ps{qc}_{dt}")
                            for h in range(c.hpc):
                                nc.tensor.matmul(o_ps[:], wot[:, h, dsl],
                                                 ct_tiles[h][:],
                                                 start=(h == 0),
                                                 stop=(h == c.hpc - 1))
                            o_sb = wp.tile([128, c.QC], F16, tag="osb", bufs=6,
                                           name=f"o_sb{qc}_{dt}")
                            if copy_split is None:
                                on_dve = dt % 2 == 1
                            else:
                                on_dve = dt < copy_split
                            if on_dve:
                                nc.vector.tensor_copy(o_sb[:], o_ps[:])
                            else:
                                nc.scalar.copy(o_sb[:], o_ps[:])
                            nc.sync.dma_start(out_d[dsl, sl], o_sb[:])
                        return emit
                    return [unit(dt) for dt in range(c.nd)]

                def out_proj_tail(qc, ct_tiles):
                    # final out-proj: 4 PSUM slots (accps + big pools) and a
                    # 2-ahead first-head window so the PE never waits on the
                    # PSUM-evacuation copies. Output DMA batched 4 dt-tiles
                    # per transfer on the SP ring (one issue slot, line-rate)
                    sl = slice(qc * c.QC, (qc + 1) * c.QC)
                    out_r = out_d.rearrange("(t p) s -> p t s", p=128)
                    o_ps = {}
                    grp = 4
                    o_sb4 = None

                    def open_dt(dt):
                        tag = "big" if dt % 2 == 0 else "accps"
                        ps = psp.tile([128, c.QC], F32, tag=tag, bufs=2,
                                      name=f"o_ps{qc}_{dt}")
                        dsl = slice(dt * 128, (dt + 1) * 128)
                        nc.tensor.matmul(ps[:], wot[:, 0, dsl], ct_tiles[0][:],
                                         start=True, stop=(c.hpc == 1))
                        o_ps[dt] = ps

                    # group bounds: big batches first, tiny last ones so
                    # the final transfer lands right after the final copy
                    cuts = list(range(grp, c.nd, grp)) + [c.nd]
                    gstart = 0
                    for dt in range(min(2, c.nd)):
                        open_dt(dt)
                    for dt in range(c.nd):
                        ps = o_ps.pop(dt)
                        dsl = slice(dt * 128, (dt + 1) * 128)
                        for h in range(1, c.hpc):
                            nc.tensor.matmul(ps[:], wot[:, h, dsl],
                                             ct_tiles[h][:],
                                             start=False, stop=(h == c.hpc - 1))
                        if dt + 2 < c.nd:
                            open_dt(dt + 2)
                        gend = min(cc for cc in cuts if cc > dt)
                        if dt == gstart:
                            o_sb4 = wp.tile([128, gend - gstart, c.QC], F16,
                                            tag="osb4", bufs=2,
                                            name=f"o_sb4_{qc}_{dt}")
                        if dt % 2 == 1:
                            nc.vector.tensor_copy(o_sb4[:, dt - gstart, :], ps[:])
                        else:
                            nc.scalar.copy(o_sb4[:, dt - gstart, :], ps[:])
                        if dt == gend - 1:
                            nc.sync.dma_start(out_r[:, gstart:gend, sl],
                                              o_sb4[:])
                            gstart = gend

                # ---------- fused pipeline ----------
                proj_chunk(0)
                adapter_kv()
                pending = []

                def filler(n):
                    for _ in range(n):
                        if pending:
                            pending.pop(0)()

                for qc in range(c.ns):
                    last = qc + 1 == c.ns
                    state, ct_tiles = {}, {}
                    sums_box = [None]
                    for h in range(c.hpc):
                        scores_head(qc, h, state, filler, sums_box)
                        if last and h + 1 < c.hpc:
                            # hide head h's combine chain under head h+1's
                            # k-loop (drained via the filler)
                            hh = h

                            def comb():
                                combine_head(qc, hh, state, ct_tiles)
                            pending.insert(0, comb)
                    while pending:
                        pending.pop(0)()
                    if not last:
                        proj_chunk(qc + 1)
                        for h in range(c.hpc):
                            combine_head(qc, h, state, ct_tiles)
                        for u in out_proj_units(qc, ct_tiles):
                            u()
                    else:
                        if c.hpc == 1:
                            combine_head(qc, 0, state, ct_tiles)
                        else:
                            combine_head(qc, c.hpc - 1, state, ct_tiles)
                        out_proj_tail(qc, ct_tiles)

    nc.compile()
    return nc


def make_in_maps(cfg, hidden_states, Wq, Wk, Wv, Wo, adaption_prompt,
                 adaption_gate, position_ids):
    """Host-side sharding: slice/transpose/cast per core + RoPE tables."""
    c = cfg
    x = np.asarray(hidden_states, np.float32)[0]          # [s, d]
    xt = np.ascontiguousarray(x.T).astype(NP_F16)         # [d, s]
    ap = np.asarray(adaption_prompt, np.float32)[0]       # [L, d]
    apt = np.ascontiguousarray(ap.T).astype(NP_F16)       # [d, L]
    gate = float(np.asarray(adaption_gate).reshape(-1)[0])
    aptv = np.ascontiguousarray(gate * ap.T).astype(NP_F16)
    # RoPE tables in the [hd, s] transposed layout; sin pre-signed.
    pos = np.asarray(position_ids).reshape(-1).astype(np.float64)  # [s]
    inv = 1.0 / (c.rope_base ** (np.arange(0, c.hd, 2, dtype=np.float64) / c.hd))
    f = inv[:, None] * pos[None, :]                       # [hd/2, s]
    cos_t = np.concatenate([np.cos(f), np.cos(f)], axis=0).astype(NP_F16)
    sv = np.sin(f)
    # halves swapped: rows 0:64 multiply q[0:64] (+sin, lands in dst[64:]),
    # rows 64:128 multiply q[64:128] (-sin, lands in dst[0:64])
    sin_t = np.concatenate([sv, -sv], axis=0).astype(NP_F16)
    in_maps = []
    for i in range(c.n_cores):
        rs = slice(i * c.dh, (i + 1) * c.dh)
        wq_t = np.asarray(Wq, np.float32)[rs, :].T.astype(NP_F16)   # [d, dh]
        wk_t = np.asarray(Wk, np.float32)[rs, :].T.astype(NP_F16)
        wqk = np.ascontiguousarray(np.stack([wq_t, wk_t], axis=1))  # [d, 2, dh]
        in_maps.append({
            "xt": xt,
            "wqk": wqk,
            "wvt": np.ascontiguousarray(np.asarray(Wv, np.float32)[rs, :].T).astype(NP_F16),
            "wot": np.ascontiguousarray(np.asarray(Wo, np.float32)[:, rs].T).astype(NP_F16),
            "apt": apt,
            "aptv": aptv,
            "cost": cos_t,
            "sint": sin_t,
        })
    return in_maps


def assemble_output(cfg, results):
    acc = np.zeros((cfg.d, cfg.s), np.float32)
    for r in results:
        acc += np.asarray(r["out"], np.float32)           # per-core partial [d, s]
    return np.ascontiguousarray(acc.T)[None]              # [1, s, d]


_NC_CACHE = {}


def run(inputs, cfg=None, trace=False):
    cfg = cfg or Cfg()
    key = (cfg.s, cfg.d, cfg.L, cfg.n_cores, cfg.n_heads)
    if key not in _NC_CACHE:
        _NC_CACHE[key] = build(cfg)
    nc = _NC_CACHE[key]
    in_maps = make_in_maps(cfg, **inputs)
    res = run_bass_kernel_spmd(nc, in_maps, core_ids=list(range(cfg.n_cores)),
                               trace=trace)
    out = assemble_output(cfg, res.results)
    return out, res


def kernel(**inputs) -> np.ndarray:
    out, _ = run(inputs)
    return out.astype(np.float32)
